# revision 1
# baseline (speedup 1.0000x reference)
"""ECGformer forward pass on 8 TRN2 NeuronCores, data-parallel over batch.

Layout strategy per core (1 batch element):
- Residual stream h: token-major fp32, [128, 9*256] (q-tile j at cols 256j;
  tile 8 holds only token 1024 in row 0).
- LayerNorm in token-major (per-partition stats), affine applied during the
  PE-transpose evacuation (feature-major, per-partition scale/bias on ACT).
- Attention in [k, q] feature-major: energy^T = K^T-slice.T @ Q^T via K=32
  row-tiled matmuls; softmax denominator comes free from a 16.0-valued
  augmentation column in V (also folds the post-softmax /sqrt(E) division);
  O^T accumulated per head with M=33 matmuls.  Straggler-query energies ride
  inline right after the same-stationary main energy matmuls (LDWEIGHTS
  dedupe).  PV PSUM is evacuated raw with one wide DVE copy per pair so the
  accumulator frees early; the reciprocal/broadcast/multiply normalization
  chain then runs off SBUF without blocking the next pair.
- Per-tile pipelining: residual adds, pos-add and LN stats are issued per
  q-tile so DVE/ACT epilogue work overlaps PE matmuls of later tiles.
- All matmul operands bf16 (fp32 PSUM accumulation); weights fed bf16 from
  host, permuted so attention head h lives in "slot" s with 32-aligned rows.
- Biases: bq/bk added per-partition during Q^T/K^T evac; bv+bo folded into a
  host-precomputed row added via a K=1 ones-matmul into the Wo PSUM; b1 added
  per-partition in the gelu evac; b2/b_emb/bc1/bc2 via K=1 ones-matmuls.
"""

import os
import sys

import numpy as np

try:
    import ml_dtypes
except ImportError:  # pragma: no cover
    ml_dtypes = None

TRN_REPO = "/opt/trn_rl_repo"

B, N, C = 8, 1024, 12
E, H, HD, L, FF, NCLS = 256, 8, 32, 4, 1024, 5
S = N + 1          # 1025 tokens
NQ = 9             # token tiles (8 full + 1 single-row)
NKT = 9            # k tiles in attention
EPS = 1e-5
SIG = [0, 4, 1, 5, 2, 6, 3, 7]   # slot s holds head SIG[s]
DIV = 16.0         # sqrt(E); folded into the V augmentation column

_CACHE = {}
DVEK = {int(v) for v in os.environ.get("DVEK", "0,2,4,6").split(",") if v != ""}


EXPA1, EXPA2, EXPA3 = 0.0625147850914497, 0.0019667051147439383, 4.0039417979884645e-05


def _register_exp_ops():
    import re
    from concourse import dve_ops as D
    from concourse.dve_spec import Spec, Src0, C0, C1, C2, One, sq
    if any(op.name == "ANT_EXPP" for op in D.OPS):
        return
    expp = D.DveOp(
        name="ANT_EXPP",
        spec=Spec(
            body=((C0 * Src0 + C1) * Src0 + C2) * Src0 + One,
            reference=lambda in0, s0, s1, imm2:
                ((s0 * in0 + s1) * in0 + imm2) * in0 + 1.0,
        ),
        subdim=False, uops_sha={})
    sq16 = D.DveOp(
        name="ANT_SQ16",
        spec=Spec(body=sq(sq(sq(sq(Src0)))),
                  reference=lambda in0: (((in0 ** 2) ** 2) ** 2) ** 2),
        subdim=False, uops_sha={})
    for op in (expp, sq16):
        D.OPS.append(op)
        D.CUSTOM_DVE_SPECS[op.name] = op.spec
        D._SUB_OPCODE_FOR_NAME[op.name] = D._CUSTOM_DVE_ROW_BASE + len(D.OPS) - 1
        assert D._SUB_OPCODE_FOR_NAME[op.name] < 0x20
        for ver in ("v3", "v4"):
            try:
                op.compile(ver)
            except ValueError as e:
                m = re.search(r"\(" + ver + r": ([0-9a-f]+)", str(e))
                op.uops_sha[ver] = m.group(1)
        op.compile("v3")
    return expp, sq16


def _build():
    sys.path.insert(0, TRN_REPO)
    import concourse.tile as tile
    from concourse import mybir, bacc
    from concourse.masks import make_identity
    from concourse import dve_ops as D
    _register_exp_ops()
    EXPP = next(op for op in D.OPS if op.name == "ANT_EXPP")
    SQ16 = next(op for op in D.OPS if op.name == "ANT_SQ16")

    F32 = mybir.dt.float32
    BF16 = mybir.dt.bfloat16
    AF = mybir.ActivationFunctionType
    OP = mybir.AluOpType

    nc = bacc.Bacc("TRN2", target_bir_lowering=False, debug=False, num_devices=8)

    # ---------------- DRAM parameters ----------------
    def din(name, shape, dt=F32):
        return nc.dram_tensor(name, shape, dt, kind="ExternalInput").ap()

    x_d = din("x", [N, C])
    pos_d = din("pos_tm", [128, NQ * E])
    cls_d = din("cls_row", [1, E])
    wemb_d = din("wemb", [C, E], BF16)
    bemb_d = din("bemb_row", [1, E], BF16)
    gemb_d = din("gemb_bc", [128, E])
    beemb_d = din("beemb_bc", [128, E])

    wq_d, wk_d, wv_d, wo_d, w1_d, w2_d = [], [], [], [], [], []
    ln1s_d, ln1b_d, ln2s_d, ln2b_d = [], [], [], []
    bqf_d, bkf_d, b1f_d, wob_d, b2r_d = [], [], [], [], []
    for l in range(L):
        wq_d.append(din(f"wq{l}", [128, 2 * E], BF16))
        wk_d.append(din(f"wk{l}", [128, 2 * E], BF16))
        wv_d.append(din(f"wv{l}", [128, 2 * E], BF16))
        wo_d.append(din(f"wo{l}", [96, 8 * E], BF16))
        w1_d.append(din(f"w1{l}", [128, 2 * FF], BF16))
        w2_d.append(din(f"w2{l}", [128, 8 * E], BF16))
        ln1s_d.append(din(f"ln1s{l}", [128, 2]))
        ln1b_d.append(din(f"ln1b{l}", [128, 2]))
        ln2s_d.append(din(f"ln2s{l}", [128, 2]))
        ln2b_d.append(din(f"ln2b{l}", [128, 2]))
        bqf_d.append(din(f"bqf{l}", [128, 2]))
        bkf_d.append(din(f"bkf{l}", [128, 2]))
        b1f_d.append(din(f"b1f{l}", [128, 8]))
        wob_d.append(din(f"wob{l}", [1, E], BF16))
        b2r_d.append(din(f"b2r{l}", [1, E], BF16))
    wc1_d = din("wc1", [128, 2 * E], BF16)
    bc1_d = din("bc1r", [1, E], BF16)
    lncg_d = din("lncg", [1, E])
    lncb_d = din("lncb", [1, E])
    wc2_d = din("wc2", [128, 2 * NCLS], BF16)
    bc2_d = din("bc2r", [1, NCLS], BF16)

    out_d = nc.dram_tensor("out", [1, NCLS], F32, kind="ExternalOutput").ap()

    with tile.TileContext(nc) as tc:
        cp = tc.alloc_tile_pool(name="consts", bufs=1)
        ap_ = tc.alloc_tile_pool(name="acts", bufs=1)
        ptp = tc.alloc_tile_pool(name="ptp", bufs=4)
        bcp = tc.alloc_tile_pool(name="bcp", bufs=2)
        dnp = tc.alloc_tile_pool(name="dnp", bufs=2)
        psM = tc.alloc_tile_pool(name="psM", bufs=1, space="PSUM")
        psE = tc.alloc_tile_pool(name="psE", bufs=3, space="PSUM")
        ps2 = tc.alloc_tile_pool(name="ps2", bufs=2, space="PSUM")

        # ---------------- constants into SBUF ----------------
        def load(name, dram, shape, dt=F32):
            t = cp.tile(shape, dt, tag=name)
            nc.sync.dma_start(t[:], dram[:])
            return t

        pos_sb = load("pos", pos_d, [128, NQ * E])
        wemb_sb = load("wemb", wemb_d, [C, E], BF16)
        bemb_sb = load("bemb", bemb_d, [1, E], BF16)
        gemb_sb = load("gemb", gemb_d, [128, E])
        beemb_sb = load("beemb", beemb_d, [128, E])
        wq_sb = [load(f"wq{l}", wq_d[l], [128, 2 * E], BF16) for l in range(L)]
        wk_sb = [load(f"wk{l}", wk_d[l], [128, 2 * E], BF16) for l in range(L)]
        wv_sb = [load(f"wv{l}", wv_d[l], [128, 2 * E], BF16) for l in range(L)]
        wo_sb = [load(f"wo{l}", wo_d[l], [96, 8 * E], BF16) for l in range(L)]
        w1_sb = [load(f"w1{l}", w1_d[l], [128, 2 * FF], BF16) for l in range(L)]
        w2_sb = [load(f"w2{l}", w2_d[l], [128, 8 * E], BF16) for l in range(L)]
        ln1s_sb = [load(f"ln1s{l}", ln1s_d[l], [128, 2]) for l in range(L)]
        ln1b_sb = [load(f"ln1b{l}", ln1b_d[l], [128, 2]) for l in range(L)]
        ln2s_sb = [load(f"ln2s{l}", ln2s_d[l], [128, 2]) for l in range(L)]
        ln2b_sb = [load(f"ln2b{l}", ln2b_d[l], [128, 2]) for l in range(L)]
        bqf_sb = [load(f"bqf{l}", bqf_d[l], [128, 2]) for l in range(L)]
        bkf_sb = [load(f"bkf{l}", bkf_d[l], [128, 2]) for l in range(L)]
        b1f_sb = [load(f"b1f{l}", b1f_d[l], [128, 8]) for l in range(L)]
        wob_sb = [load(f"wob{l}", wob_d[l], [1, E], BF16) for l in range(L)]
        b2r_sb = [load(f"b2r{l}", b2r_d[l], [1, E], BF16) for l in range(L)]
        wc1_sb = load("wc1", wc1_d, [128, 2 * E], BF16)
        bc1_sb = load("bc1", bc1_d, [1, E], BF16)
        lncg_sb = load("lncg", lncg_d, [1, E])
        lncb_sb = load("lncb", lncb_d, [1, E])
        wc2_sb = load("wc2", wc2_d, [128, 2 * NCLS], BF16)
        bc2_sb = load("bc2", bc2_d, [1, NCLS], BF16)

        ident = cp.tile([128, 128], BF16, tag="ident")
        make_identity(nc, ident[:])
        ones_row = cp.tile([1, 128], BF16, tag="ones_row")
        nc.vector.memset(ones_row[:], 1.0)
        ones_col = cp.tile([128, 1], BF16, tag="ones_col")
        nc.vector.memset(ones_col[:], 1.0)

        # residual stream
        h = cp.tile([128, NQ * E], F32, tag="h")
        nc.vector.memset(h[:, 8 * E:], 0.0)

        # ---------------- embedding ----------------
        xT = ap_.tile([C, N], F32, tag="xT")
        nc.sync.dma_start(xT[:], x_d[:].rearrange("n c -> c n"))
        xTb = ap_.tile([C, N], BF16, tag="xTb")
        nc.vector.tensor_copy(xTb[:], xT[:])

        embg = ap_.tile([128, 8 * E], F32, tag="gt")   # shares slot with GT
        st6 = ap_.tile([128, 6 * NQ], F32, tag="st6")
        agg = ap_.tile([128, 2 * NQ], F32, tag="agg")
        rstd = ap_.tile([128, NQ], F32, tag="rstd")
        nmr = ap_.tile([128, NQ], F32, tag="nmr")
        tmpa = ap_.tile([128, NQ], F32, tag="tmpa")

        for j in range(8):
            ep = ps2.tile([128, E], F32, tag="misc")
            nc.tensor.matmul(ep[:], xTb[:, 128 * j:128 * (j + 1)], wemb_sb[:],
                             start=True, stop=False)
            nc.tensor.matmul(ep[:], ones_row[0:1, 0:128], bemb_sb[:],
                             start=False, stop=True)
            # LN stats for this tile
            nc.vector.bn_stats(st6[:, 6 * j:6 * j + 6], ep[:])
            nc.vector.bn_aggr(agg[:, 2 * j:2 * j + 2], st6[:, 6 * j:6 * j + 6])
            # stash raw emb (fp32) temporarily in embg
            nc.vector.tensor_copy(embg[:, E * j:E * (j + 1)], ep[:])
        agg3 = agg[:].rearrange("p (j t) -> p t j", t=2)
        nc.vector.tensor_scalar_add(tmpa[:, 0:8], agg3[:, 1:2, 0:8], EPS)
        nc.vector.reciprocal_approx_fast(rstd[:, 0:8], tmpa[:, 0:8])
        nc.scalar.activation(rstd[:, 0:8], rstd[:, 0:8], AF.Sqrt)
        nc.vector.tensor_tensor(out=nmr[:, 0:8], in0=agg3[:, 0:1, 0:8],
                                in1=rstd[:, 0:8], op=OP.mult)
        nc.vector.tensor_scalar_mul(nmr[:, 0:8], nmr[:, 0:8], -1.0)
        for j in range(8):
            sl = slice(E * j, E * (j + 1))
            # (x - m) * rstd, then * g + b (broadcast consts), then gelu
            nc.vector.tensor_scalar(out=embg[:, sl], in0=embg[:, sl],
                                    scalar1=agg3[:, 0:1, j:j + 1],
                                    scalar2=rstd[:, j:j + 1],
                                    op0=OP.subtract, op1=OP.mult)
            nc.vector.tensor_tensor(out=embg[:, sl], in0=embg[:, sl],
                                    in1=gemb_sb[:], op=OP.mult)
            nc.vector.tensor_tensor(out=embg[:, sl], in0=embg[:, sl],
                                    in1=beemb_sb[:], op=OP.add)
            nc.scalar.activation(embg[:, sl], embg[:, sl], AF.Gelu)
        # shift into h: h token 128j+p+1 <- emb token 128j+p
        for j in range(NQ):
            if j < 8:
                nc.sync.dma_start(h[1:128, E * j:E * j + E],
                                  embg[0:127, E * j:E * j + E])
            if j >= 1:
                nc.sync.dma_start(h[0:1, E * j:E * j + E],
                                  embg[127:128, E * (j - 1):E * j])
        nc.sync.dma_start(h[0:1, 0:E], cls_d[:])

        # ---------------- helpers ----------------
        def ln_stats(j):
            """per-tile LN stats (DVE) — issued early for pipelining."""
            nc.vector.bn_stats(st6[:, 6 * j:6 * j + 6],
                               h[:, E * j:E * (j + 1)])
            nc.vector.bn_aggr(agg[:, 2 * j:2 * j + 2],
                              st6[:, 6 * j:6 * j + 6])

        def layer_norm(lns, lnb, ytA, ytB, stats_done):
            """token-major LN of h -> feature-major bf16 [128,1025] x2.

            If stats_done, bn_stats/bn_aggr were already issued per-tile.
            """
            y0 = ap_.tile([128, NQ * E], BF16, tag="y0")
            if not stats_done:
                for j in range(NQ):
                    ln_stats(j)
            a3 = agg[:].rearrange("p (j t) -> p t j", t=2)
            nc.vector.tensor_scalar_add(tmpa[:], a3[:, 1:2, :], EPS)
            nc.vector.reciprocal_approx_fast(rstd[:], tmpa[:])
            nc.scalar.activation(rstd[:], rstd[:], AF.Sqrt)
            for j in range(NQ):
                # y0 = (h - mean) * rstd on DVE (keeps ACT free for the
                # transpose evacuations)
                nc.vector.tensor_scalar(out=y0[:, E * j:E * (j + 1)],
                                        in0=h[:, E * j:E * (j + 1)],
                                        scalar1=a3[:, 0:1, j:j + 1],
                                        scalar2=rstd[:, j:j + 1],
                                        op0=OP.subtract, op1=OP.mult)
            # transpose + affine evac
            for t, yt in ((0, ytA), (1, ytB)):
                for jb in range(3):
                    js = list(range(4 * jb, min(4 * jb + 4, NQ)))
                    tp = ps2.tile([128, 512], BF16, tag="misc")
                    for i, j in enumerate(js):
                        nc.tensor.transpose(
                            tp[:, 128 * i:128 * (i + 1)],
                            y0[:, E * j + 128 * t:E * j + 128 * t + 128],
                            ident[:])
                    w = 128 * len(js) if jb < 2 else 1
                    nc.scalar.activation(yt[:, 512 * jb:512 * jb + w],
                                         tp[:, 0:w], AF.Identity,
                                         bias=lnb[:, t:t + 1],
                                         scale=lns[:, t:t + 1])

        def project_qk(w_sb, bias_fm, ys, qtA, qtB):
            """yT @ W -> feature-major [2][128,1025] bf16 with bias."""
            for m, qt in ((0, qtA), (1, qtB)):
                for c0, cw in ((0, 512), (512, 512), (1024, 1)):
                    pp = ps2.tile([128, 512], F32, tag="misc")
                    for t in range(2):
                        nc.tensor.matmul(
                            pp[:, 0:cw],
                            w_sb[:, E * t + 128 * m:E * t + 128 * m + 128],
                            ys[t][:, c0:c0 + cw],
                            start=(t == 0), stop=(t == 1))
                    nc.scalar.activation(qt[:, c0:c0 + cw], pp[:, 0:cw],
                                         AF.Identity,
                                         bias=bias_fm[:, m:m + 1], scale=1.0)

        # ---------------- transformer layers ----------------
        for l in range(L):
            if l == 0:
                # h += pos; LN1 stats inline (later layers do this per-tile
                # inside the MLP epilogue)
                nc.gpsimd.tensor_tensor(out=h[:], in0=h[:], in1=pos_sb[:],
                                        op=OP.add)
                for j in range(NQ):
                    ln_stats(j)
            # ---- attention ----
            ytA = ap_.tile([128, S], BF16, tag="ytA")
            ytB = ap_.tile([128, S], BF16, tag="ytB")
            layer_norm(ln1s_sb[l], ln1b_sb[l], ytA, ytB, stats_done=True)

            qtA = ap_.tile([128, S], BF16, tag="qtA")
            qtB = ap_.tile([128, S], BF16, tag="qtB")
            ktA = ap_.tile([128, S], BF16, tag="ktA")
            ktB = ap_.tile([128, S], BF16, tag="ktB")
            project_qk(wq_sb[l], bqf_sb[l], (ytA, ytB), qtA, qtB)
            project_qk(wk_sb[l], bkf_sb[l], (ytA, ytB), ktA, ktB)

            # V token-major with 16.0 augmentation columns
            vsb = ap_.tile([128, NKT * 264], BF16, tag="vsb")
            v4 = vsb[:].rearrange("p (k s e) -> p k s e", k=NKT, s=8)
            nc.vector.memset(v4[:, :, :, 32:33], DIV)
            for kt in range(NKT):
                mw = 128 if kt < 8 else 1
                vp = ps2.tile([128, E], F32, tag="misc")
                for t in range(2):
                    nc.tensor.matmul(
                        vp[0:mw, :],
                        (ytA if t == 0 else ytB)[:, 128 * kt:128 * kt + mw],
                        wv_sb[l][:, E * t:E * (t + 1)],
                        start=(t == 0), stop=(t == 1))
                nc.vector.tensor_copy(
                    v4[0:mw, kt, :, 0:32],
                    vp[0:mw, :].rearrange("p (s d) -> p s d", s=8))

            # attention output per slot (normalized O^T, bf16)
            otp = []
            for s in range(8):
                ot_s = ap_.tile([97, S], BF16, tag=f"otp{s}")
                otp.append(ot_s)

            def slot_rows(s):
                qt = qtA if s < 4 else qtB
                kt_t = ktA if s < 4 else ktB
                rp = 32 * (s % 4)
                return qt[rp:rp + 32, :], kt_t[rp:rp + 32, :], rp

            stot = psM.tile([128, 96], F32, tag="stot")
            nc.vector.memset(stot[:], 0.0)
            pairs = [(2 * p, 2 * p + 1) for p in range(4)]
            for sA, sB in pairs:
                pv = psM.tile([128, 1024], F32, tag="pv")
                group = []
                for s in (sA, sB):
                    qr, kr, rp = slot_rows(s)
                    ob = 0 if s == sA else 64
                    group.append((s, qr, kr, rp, ob))
                for kt in range(NKT):
                    mw = 128 if kt < 8 else 1
                    for c0 in (0, 512):
                        engs = []
                        for s, qr, kr, rp, ob in group:
                            eps_t = psE.tile([128, 512], F32, tag="eng")
                            nc.tensor.matmul(
                                eps_t[0:mw, :],
                                kr[:, 128 * kt:128 * kt + mw],
                                qr[:, c0:c0 + 512],
                                start=True, stop=True,
                                tile_position=(rp, 0))
                            engs.append(eps_t)
                        pts_c = []
                        for (s, qr, kr, rp, ob), eps_t in zip(group, engs):
                            ptt = ptp.tile([128, 512], BF16, tag="pt")
                            # slot-parity engine split: odd slots take DVE for
                            # kts in DVEK so both PV inputs arrive together
                            if s % 2 == 1 and kt in DVEK:
                                etmp = dnp.tile([128, 512], F32, tag="etmp")
                                nc.vector._custom_dve(
                                    EXPP, out=etmp[:], in0=eps_t[:],
                                    s0=EXPA3, s1=EXPA2, imm2=EXPA1)
                                nc.vector._custom_dve(SQ16, out=ptt[:],
                                                      in0=etmp[:])
                            else:
                                nc.scalar.activation(ptt[0:mw, :],
                                                     eps_t[0:mw, :], AF.Exp)
                            pts_c.append(ptt)
                        for (s, qr, kr, rp, ob), ptt in zip(group, pts_c):
                            nc.tensor.matmul(
                                pv[ob:ob + 33, c0:c0 + 512],
                                vsb[0:mw, 264 * kt + 33 * s:
                                    264 * kt + 33 * s + 33],
                                ptt[0:mw, :],
                                start=(kt == 0), stop=(kt == 8),
                                skip_group_check=True,
                                tile_position=(0, ob))
                # straggler-query energies + softmax + PV per slot
                for s, qr, kr, rp, ob in group:
                    for kt in range(NKT):
                        mw = 128 if kt < 8 else 1
                        nc.tensor.matmul(
                            stot[0:mw, 9 * s + kt:9 * s + kt + 1],
                            kr[:, 128 * kt:128 * kt + mw],
                            qr[:, 1024:1025],
                            start=True, stop=True,
                            tile_position=(rp, 0))
                    ptsl = ptp.tile([128, NKT], BF16, tag="pts")
                    nc.scalar.activation(ptsl[:], stot[:, 9 * s:9 * s + NKT],
                                         AF.Exp)
                    for kt in range(NKT):
                        mw = 128 if kt < 8 else 1
                        nc.tensor.matmul(
                            stot[ob:ob + 33, 80 + s:81 + s],
                            vsb[0:mw, 264 * kt + 33 * s:
                                264 * kt + 33 * s + 33],
                            ptsl[0:mw, kt:kt + 1],
                            start=(kt == 0), stop=(kt == 8),
                            skip_group_check=True,
                            tile_position=(0, ob))
                # Evacuate the whole PV block (rows 0..96) raw with one wide
                # copy: frees the pv PSUM accumulator early so the next
                # pair's PV can start; the normalization chain below then
                # reads SBUF only.
                pvr = dnp.tile([97, S], F32, tag="pvr")
                nc.vector.tensor_copy(pvr[0:33, 0:1024], pv[0:33, :])
                nc.vector.tensor_copy(pvr[64:97, 0:1024], pv[64:97, :])
                for s, qr_, kr_, rp_, ob in group:
                    nc.vector.tensor_copy(pvr[ob:ob + 33, 1024:1025],
                                          stot[ob:ob + 33, 80 + s:81 + s])
                for s, qr_, kr_, rp_, ob in group:
                    dr = ob + 32
                    dnQ = dnp.tile([1, S], F32, tag="dnQ")
                    nc.gpsimd.dma_start(dnQ[0:1, :], pvr[dr:dr + 1, :])
                    rcp = dnp.tile([1, S], F32, tag="rcp")
                    nc.vector.reciprocal_approx_fast(rcp[:], dnQ[:])
                    # broadcast across all 96 rows (dst base partition 0);
                    # the SBUF-SBUF multiply below then uses matching row
                    # slices so base partitions agree
                    bc = bcp.tile([96, S], F32, tag="bc")
                    nc.gpsimd.partition_broadcast(bc[:], rcp[:])
                    nc.vector.tensor_tensor(out=otp[s][ob:ob + 32, :],
                                            in0=pvr[ob:ob + 32, :],
                                            in1=bc[ob:ob + 32, :], op=OP.mult)

            # Wo projection + residual (+ bo + bv@Wo row); per-tile LN2
            # stats issued right after each tile's residual add.
            for qt_i in range(NQ):
                mw = 128 if qt_i < 8 else 1
                wp = ps2.tile([128, E], F32, tag="misc")
                wp2 = psE.tile([128, E], F32, tag="eng")
                for s in range(0, 8, 2):
                    nc.tensor.matmul(
                        wp[0:mw, :],
                        otp[s][0:32, 128 * qt_i:128 * qt_i + mw],
                        wo_sb[l][0:32, E * s:E * (s + 1)],
                        start=(s == 0), stop=False,
                        tile_position=(0, 0))
                    nc.tensor.matmul(
                        wp2[0:mw, :],
                        otp[s + 1][64:96, 128 * qt_i:128 * qt_i + mw],
                        wo_sb[l][64:96, E * (s + 1):E * (s + 2)],
                        start=(s == 0), stop=(s == 6),
                        tile_position=(64, 0))
                nc.tensor.matmul(wp[0:mw, :], ones_row[0:1, 0:mw],
                                 wob_sb[l][:], start=False, stop=True,
                                 tile_position=(0, 0))
                sl = slice(E * qt_i, E * (qt_i + 1))
                nc.vector.tensor_tensor(out=h[0:mw, sl], in0=h[0:mw, sl],
                                        in1=wp[0:mw, :], op=OP.add)
                nc.vector.tensor_tensor(out=h[0:mw, sl], in0=h[0:mw, sl],
                                        in1=wp2[0:mw, :], op=OP.add)
                ln_stats(qt_i)

            # ---- MLP ----
            ytA = ap_.tile([128, S], BF16, tag="ytA")
            ytB = ap_.tile([128, S], BF16, tag="ytB")
            layer_norm(ln2s_sb[l], ln2b_sb[l], ytA, ytB, stats_done=True)

            # W1 chunk-major so W2 tiles start while later chunks project
            gt = ap_.tile([128, 8 * S], BF16, tag="gt")
            for c0, cw in ((0, 512), (512, 512), (1024, 1)):
                for f in range(8):
                    gp = ps2.tile([128, 512], F32, tag="misc")
                    for t in range(2):
                        nc.tensor.matmul(
                            gp[:, 0:cw],
                            w1_sb[l][:, FF * t + 128 * f:FF * t + 128 * f + 128],
                            (ytA if t == 0 else ytB)[:, c0:c0 + cw],
                            start=(t == 0), stop=(t == 1))
                    nc.scalar.activation(gt[:, S * f + c0:S * f + c0 + cw],
                                         gp[:, 0:cw], AF.Gelu,
                                         bias=b1f_sb[l][:, f:f + 1], scale=1.0)
            for qt_i in range(NQ):
                mw = 128 if qt_i < 8 else 1
                wp = ps2.tile([128, E], F32, tag="misc")
                for f in range(8):
                    nc.tensor.matmul(wp[0:mw, :],
                                     gt[:, S * f + 128 * qt_i:
                                        S * f + 128 * qt_i + mw],
                                     w2_sb[l][:, E * f:E * (f + 1)],
                                     start=(f == 0), stop=False)
                nc.tensor.matmul(wp[0:mw, :], ones_row[0:1, 0:mw],
                                 b2r_sb[l][:], start=False, stop=True)
                sl = slice(E * qt_i, E * (qt_i + 1))
                nc.vector.tensor_tensor(out=h[0:mw, sl], in0=h[0:mw, sl],
                                        in1=wp[0:mw, :], op=OP.add)
                if l < L - 1:
                    # next layer's pos-add + LN1 stats, per-tile (gpsimd add
                    # keeps DVE free; stats follow on DVE)
                    nc.gpsimd.tensor_tensor(out=h[0:mw, sl], in0=h[0:mw, sl],
                                            in1=pos_sb[0:mw, sl], op=OP.add)
                    ln_stats(qt_i)

        # ---------------- classifier ----------------
        hbf = ap_.tile([128, NQ * E], BF16, tag="hbf")
        nc.vector.tensor_copy(hbf[:], h[:])
        mp = ps2.tile([1, E], F32, tag="misc")
        for j in range(8):
            nc.tensor.matmul(mp[:], ones_col[:], hbf[:, E * j:E * (j + 1)],
                             start=(j == 0), stop=False)
        nc.tensor.matmul(mp[:], ones_col[0:1, :], hbf[0:1, 8 * E:9 * E],
                         start=False, stop=True)
        pbf = dnp.tile([1, E], BF16, tag="pbf")
        nc.scalar.activation(pbf[:], mp[:], AF.Identity, bias=0.0,
                             scale=1.0 / S)
        # p @ Wc1 + bc1
        ptp_ps = ps2.tile([128, 512], BF16, tag="misc")
        pT = dnp.tile([128, 2], BF16, tag="pT")
        for t in range(2):
            nc.tensor.transpose(ptp_ps[:, 128 * t:128 * t + 1],
                                pbf[0:1, 128 * t:128 * (t + 1)],
                                ident[0:1, 0:1])
            nc.vector.tensor_copy(pT[:, t:t + 1],
                                  ptp_ps[:, 128 * t:128 * t + 1])
        c1p = ps2.tile([1, E], F32, tag="misc")
        for t in range(2):
            nc.tensor.matmul(c1p[:], pT[:, t:t + 1],
                             wc1_sb[:, E * t:E * (t + 1)],
                             start=(t == 0), stop=False)
        nc.tensor.matmul(c1p[:], ones_row[0:1, 0:1], bc1_sb[:],
                         start=False, stop=True)
        # LN over the [1, E] row
        p2 = dnp.tile([1, E], F32, tag="p2")
        nc.vector.tensor_copy(p2[:], c1p[:])
        st1 = dnp.tile([1, 6], F32, tag="st1")
        ag1 = dnp.tile([1, 2], F32, tag="ag1")
        nc.vector.bn_stats(st1[:], p2[:])
        nc.vector.bn_aggr(ag1[:], st1[:])
        r1 = dnp.tile([1, 2], F32, tag="r1")
        nc.vector.tensor_scalar_add(r1[:, 0:1], ag1[:, 1:2], EPS)
        nc.vector.reciprocal_approx_fast(r1[:, 0:1], r1[:, 0:1])
        nc.scalar.activation(r1[:, 0:1], r1[:, 0:1], AF.Sqrt)
        nc.vector.tensor_scalar(out=p2[:], in0=p2[:], scalar1=ag1[:, 0:1],
                                scalar2=r1[:, 0:1], op0=OP.subtract,
                                op1=OP.mult)
        nc.vector.tensor_tensor(out=p2[:], in0=p2[:], in1=lncg_sb[:],
                                op=OP.mult)
        nc.vector.tensor_tensor(out=p2[:], in0=p2[:], in1=lncb_sb[:],
                                op=OP.add)
        p2b = dnp.tile([1, E], BF16, tag="p2b")
        nc.vector.tensor_copy(p2b[:], p2[:])
        p2T = dnp.tile([128, 2], BF16, tag="p2T")
        for t in range(2):
            tp2 = ps2.tile([128, 512], BF16, tag="misc")
            nc.tensor.transpose(tp2[:, 0:1], p2b[0:1, 128 * t:128 * (t + 1)],
                                ident[0:1, 0:1])
            nc.vector.tensor_copy(p2T[:, t:t + 1], tp2[:, 0:1])
        op_ = ps2.tile([1, NCLS], F32, tag="misc")
        for t in range(2):
            nc.tensor.matmul(op_[:], p2T[:, t:t + 1],
                             wc2_sb[:, NCLS * t:NCLS * (t + 1)],
                             start=(t == 0), stop=False)
        nc.tensor.matmul(op_[:], ones_row[0:1, 0:1], bc2_sb[:],
                         start=False, stop=True)
        osb = dnp.tile([1, NCLS], F32, tag="osb")
        nc.vector.tensor_copy(osb[:], op_[:])
        nc.sync.dma_start(out_d[:], osb[:])

        for _p in (ps2, psE, psM, dnp, bcp, ptp, ap_, cp):
            _p.release()

    nc.compile()
    return nc


def _prep_shared(inputs):
    """Host-side weight preparation (shared across cores)."""
    bf16 = ml_dtypes.bfloat16
    f32 = np.float32
    g = {k: np.asarray(v, dtype=f32) for k, v in inputs.items()}
    d = {}

    pos_tm = np.zeros((128, NQ * E), f32)
    for j in range(NQ):
        n = 128 if j < 8 else 1
        pos_tm[0:n, E * j:E * (j + 1)] = g["pos"][128 * j:128 * j + n]
    d["pos_tm"] = pos_tm
    d["cls_row"] = g["cls_token"].reshape(1, E)
    d["wemb"] = g["W_emb"].astype(bf16)
    d["bemb_row"] = g["b_emb"].reshape(1, E).astype(bf16)
    d["gemb_bc"] = np.broadcast_to(g["g_emb"], (128, E)).copy()
    d["beemb_bc"] = np.broadcast_to(g["be_emb"], (128, E)).copy()

    perm = np.concatenate([np.arange(32) + 32 * SIG[s] for s in range(8)])

    def pack_k(w):  # [256, X] -> [128, 2X]
        return np.concatenate([w[0:128], w[128:256]], axis=1)

    for l in range(L):
        d[f"wq{l}"] = pack_k(g["Wq"][l][:, perm]).astype(bf16)
        d[f"wk{l}"] = pack_k(g["Wk"][l][:, perm]).astype(bf16)
        d[f"wv{l}"] = pack_k(g["Wv"][l][:, perm]).astype(bf16)
        # wo: [96, 8*E]; slot s cols E*s hold Wo rows of head SIG[s], at
        # partition rows 0..31 for even s and 64..95 for odd s.
        wo = np.zeros((96, 8 * E), f32)
        for s in range(8):
            r0 = 0 if s % 2 == 0 else 64
            wo[r0:r0 + 32, E * s:E * (s + 1)] = \
                g["Wo"][l][32 * SIG[s]:32 * SIG[s] + 32]
        d[f"wo{l}"] = wo.astype(bf16)
        d[f"w1{l}"] = pack_k(g["W1"][l]).astype(bf16)
        w2 = np.concatenate([g["W2"][l][128 * t:128 * (t + 1)]
                             for t in range(8)], axis=1)
        d[f"w2{l}"] = w2.astype(bf16)
        d[f"ln1s{l}"] = pack_k(g["ln1_g"][l].reshape(E, 1))
        d[f"ln1b{l}"] = pack_k(g["ln1_b"][l].reshape(E, 1))
        d[f"ln2s{l}"] = pack_k(g["ln2_g"][l].reshape(E, 1))
        d[f"ln2b{l}"] = pack_k(g["ln2_b"][l].reshape(E, 1))
        d[f"bqf{l}"] = pack_k(g["bq"][l][perm].reshape(E, 1))
        d[f"bkf{l}"] = pack_k(g["bk"][l][perm].reshape(E, 1))
        d[f"b1f{l}"] = np.stack([g["b1"][l][128 * t:128 * (t + 1)]
                                 for t in range(8)], axis=1)
        d[f"wob{l}"] = (g["bo"][l] + g["bv"][l] @ g["Wo"][l]).reshape(1, E).astype(bf16)
        d[f"b2r{l}"] = g["b2"][l].reshape(1, E).astype(bf16)
    d["wc1"] = pack_k(g["Wc1"]).astype(bf16)
    d["bc1r"] = g["bc1"].reshape(1, E).astype(bf16)
    d["lncg"] = g["lnc_g"].reshape(1, E)
    d["lncb"] = g["lnc_b"].reshape(1, E)
    d["wc2"] = pack_k(g["Wc2"]).astype(bf16)
    d["bc2r"] = g["bc2"].reshape(1, NCLS).astype(bf16)
    return d


def kernel(**inputs):
    sys.path.insert(0, TRN_REPO)
    from concourse.bass_utils import run_bass_kernel_spmd

    if "nc" not in _CACHE:
        _CACHE["nc"] = _build()
    nc = _CACHE["nc"]

    shared = _prep_shared(inputs)
    x = np.asarray(inputs["x"], dtype=np.float32)
    in_maps = [dict(shared, x=np.ascontiguousarray(x[c])) for c in range(B)]
    res = run_bass_kernel_spmd(nc, in_maps, list(range(B)))
    out = np.stack([np.asarray(res.results[c]["out"]).reshape(NCLS)
                    for c in range(B)])
    return out.astype(np.float32)



# revision 7
# speedup vs baseline: 1.3189x; 1.3189x over previous
"""ECGformer forward pass on 8 TRN2 NeuronCores, data-parallel over batch.

Layout strategy per core (1 batch element):
- Residual stream h: token-major fp32, [128, 9*256].  Sequence order is
  [tokens 0..1023, cls]: the cls token lives at tile 8 row 0 (pos table is
  permuted to match on host).  Full softmax + mean-pool are permutation
  invariant, so this matches the reference's [cls, tokens] order exactly
  while letting the embedding write h tiles 0..7 in place (no shift DMAs).
- LayerNorm in token-major (per-partition stats); rstd = exp(-0.5*ln(v+eps))
  on ACT so the whole attention phase stays on the natural_log_exp table set
  (no sqrt-set thrash); affine applied during the PE-transpose evacuation.
- Attention in [k, q] feature-major: energy^T = K^T-slice.T @ Q^T via K=32
  row-tiled matmuls; softmax denominator comes free from a 16.0-valued
  augmentation column in V (also folds the post-softmax /sqrt(E) division);
  O^T accumulated per head with M=33 matmuls.  The kt=8 straggler key's four
  [1,512] energies are packed at rows 0/32/64/96 of one PSUM tile and
  exponentiated with a single ACT call.
- Weights arrive in a handful of large packed DMAs (one [128,7680] tensor
  per layer + small packed row/col tensors), issued after the x transpose so
  compute starts immediately.
- All matmul operands bf16 (fp32 PSUM accumulation); weights permuted so
  attention head h lives in "slot" s with 32-aligned rows.
"""

import os
import sys

import numpy as np

try:
    import ml_dtypes
except ImportError:  # pragma: no cover
    ml_dtypes = None

TRN_REPO = "/opt/trn_rl_repo"

B, N, C = 8, 1024, 12
E, H, HD, L, FF, NCLS = 256, 8, 32, 4, 1024, 5
S = N + 1          # 1025 tokens
NQ = 9             # token tiles (8 full + 1 single-row)
NKT = 9            # k tiles in attention
EPS = 1e-5
SIG = [0, 4, 1, 5, 2, 6, 3, 7]   # slot s holds head SIG[s]
DIV = 16.0         # sqrt(E); folded into the V augmentation column

# packed big-weight column offsets (bf16, [128, 7680] per layer)
OWQ, OWK, OWV, OWO, OW1, OW2 = 0, 512, 1024, 1536, 3584, 5632
# packed small-f32 column offsets ([128, 2896])
OPOS, OGEMB, OBEEMB, OSM = 0, 2304, 2560, 2816   # per-layer smalls at OSM+20*l
# packed bias-row offsets (bf16, [1, 2565])
RBEMB, RWOB, RB2R, RBC1, RBC2 = 0, 256, 1280, 2304, 2560

_CACHE = {}
DVEK = {int(v) for v in os.environ.get("DVEK", "0,2,4,6").split(",") if v != ""}


EXPA1, EXPA2, EXPA3 = 0.0625147850914497, 0.0019667051147439383, 4.0039417979884645e-05


def _register_exp_ops():
    import re
    from concourse import dve_ops as D
    from concourse.dve_spec import Spec, Src0, C0, C1, C2, One, sq
    if any(op.name == "ANT_EXPP" for op in D.OPS):
        return
    expp = D.DveOp(
        name="ANT_EXPP",
        spec=Spec(
            body=((C0 * Src0 + C1) * Src0 + C2) * Src0 + One,
            reference=lambda in0, s0, s1, imm2:
                ((s0 * in0 + s1) * in0 + imm2) * in0 + 1.0,
        ),
        subdim=False, uops_sha={})
    sq16 = D.DveOp(
        name="ANT_SQ16",
        spec=Spec(body=sq(sq(sq(sq(Src0)))),
                  reference=lambda in0: (((in0 ** 2) ** 2) ** 2) ** 2),
        subdim=False, uops_sha={})
    for op in (expp, sq16):
        D.OPS.append(op)
        D.CUSTOM_DVE_SPECS[op.name] = op.spec
        D._SUB_OPCODE_FOR_NAME[op.name] = D._CUSTOM_DVE_ROW_BASE + len(D.OPS) - 1
        assert D._SUB_OPCODE_FOR_NAME[op.name] < 0x20
        for ver in ("v3", "v4"):
            try:
                op.compile(ver)
            except ValueError as e:
                m = re.search(r"\(" + ver + r": ([0-9a-f]+)", str(e))
                op.uops_sha[ver] = m.group(1)
        op.compile("v3")
    return expp, sq16


def _build():
    sys.path.insert(0, TRN_REPO)
    import concourse.tile as tile
    from concourse import mybir, bacc
    from concourse.masks import make_identity
    from concourse import dve_ops as D
    _register_exp_ops()
    EXPP = next(op for op in D.OPS if op.name == "ANT_EXPP")
    SQ16 = next(op for op in D.OPS if op.name == "ANT_SQ16")

    F32 = mybir.dt.float32
    BF16 = mybir.dt.bfloat16
    AF = mybir.ActivationFunctionType
    OP = mybir.AluOpType

    nc = bacc.Bacc("TRN2", target_bir_lowering=False, debug=False, num_devices=8)

    # ---------------- DRAM parameters ----------------
    def din(name, shape, dt=F32):
        return nc.dram_tensor(name, shape, dt, kind="ExternalInput").ap()

    x_d = din("x", [N, C])
    wemb_d = din("wemb", [C, E], BF16)
    rowsb_d = din("rowsb", [1, 2565], BF16)
    rowsf_d = din("rowsf", [1, 3 * E])
    smallf_d = din("smallf", [128, 2896])
    bigw_d = [din(f"bigw{l}", [128, 7680], BF16) for l in range(L)]
    wc_d = din("wc", [128, 2 * E + 2 * NCLS], BF16)

    out_d = nc.dram_tensor("out", [1, NCLS], F32, kind="ExternalOutput").ap()

    with tile.TileContext(nc) as tc:
        cp = tc.alloc_tile_pool(name="consts", bufs=1)
        ap_ = tc.alloc_tile_pool(name="acts", bufs=1)
        ptp = tc.alloc_tile_pool(name="ptp", bufs=4)
        bcp = tc.alloc_tile_pool(name="bcp", bufs=2)
        dnp = tc.alloc_tile_pool(name="dnp", bufs=2)
        psM = tc.alloc_tile_pool(name="psM", bufs=1, space="PSUM")
        psE = tc.alloc_tile_pool(name="psE", bufs=3, space="PSUM")
        ps2 = tc.alloc_tile_pool(name="ps2", bufs=2, space="PSUM")

        # ---------------- input + constants into SBUF (x first) --------
        xT = ap_.tile([C, N], F32, tag="xT")
        nc.sync.dma_start(xT[:], x_d[:].rearrange("n c -> c n"))

        def load(name, dram, shape, dt=F32):
            t = cp.tile(shape, dt, tag=name)
            nc.sync.dma_start(t[:], dram[:])
            return t

        wemb_sb = load("wemb", wemb_d, [C, E], BF16)
        rowsb = load("rowsb", rowsb_d, [1, 2565], BF16)
        smallf = load("smallf", smallf_d, [128, 2896])
        rowsf = load("rowsf", rowsf_d, [1, 3 * E])
        bigw = [load(f"bigw{l}", bigw_d[l], [128, 7680], BF16) for l in range(L)]
        wc_sb = load("wc", wc_d, [128, 2 * E + 2 * NCLS], BF16)

        # views into the packed tensors
        bemb_sb = rowsb[0:1, RBEMB:RBEMB + E]
        wob_sb = [rowsb[0:1, RWOB + E * l:RWOB + E * (l + 1)] for l in range(L)]
        b2r_sb = [rowsb[0:1, RB2R + E * l:RB2R + E * (l + 1)] for l in range(L)]
        bc1_sb = rowsb[0:1, RBC1:RBC1 + E]
        bc2_sb = rowsb[0:1, RBC2:RBC2 + NCLS]
        cls_sb = rowsf[0:1, 0:E]
        lncg_sb = rowsf[0:1, E:2 * E]
        lncb_sb = rowsf[0:1, 2 * E:3 * E]
        pos_sb = smallf[:, OPOS:OPOS + NQ * E]
        gemb_sb = smallf[:, OGEMB:OGEMB + E]
        beemb_sb = smallf[:, OBEEMB:OBEEMB + E]
        sm = [smallf[:, OSM + 20 * l:OSM + 20 * (l + 1)] for l in range(L)]
        ln1s_sb = [sm[l][:, 0:2] for l in range(L)]
        ln1b_sb = [sm[l][:, 2:4] for l in range(L)]
        ln2s_sb = [sm[l][:, 4:6] for l in range(L)]
        ln2b_sb = [sm[l][:, 6:8] for l in range(L)]
        bqf_sb = [sm[l][:, 8:10] for l in range(L)]
        bkf_sb = [sm[l][:, 10:12] for l in range(L)]
        b1f_sb = [sm[l][:, 12:20] for l in range(L)]
        wq_sb = [bigw[l][:, OWQ:OWQ + 2 * E] for l in range(L)]
        wk_sb = [bigw[l][:, OWK:OWK + 2 * E] for l in range(L)]
        wv_sb = [bigw[l][:, OWV:OWV + 2 * E] for l in range(L)]
        wo_sb = [bigw[l][:, OWO:OWO + 8 * E] for l in range(L)]
        w1_sb = [bigw[l][:, OW1:OW1 + 2 * FF] for l in range(L)]
        w2_sb = [bigw[l][:, OW2:OW2 + 8 * E] for l in range(L)]
        wc1_sb = wc_sb[:, 0:2 * E]
        wc2_sb = wc_sb[:, 2 * E:2 * E + 2 * NCLS]

        ident = cp.tile([128, 128], BF16, tag="ident")
        make_identity(nc, ident[:])
        epsb = cp.tile([128, 1], F32, tag="epsb")
        nc.vector.memset(epsb[:], EPS)
        ones_row = cp.tile([1, 128], BF16, tag="ones_row")
        nc.vector.memset(ones_row[:], 1.0)
        ones_col = cp.tile([128, 1], BF16, tag="ones_col")
        nc.vector.memset(ones_col[:], 1.0)

        # residual stream
        h = cp.tile([128, NQ * E], F32, tag="h")
        nc.vector.memset(h[:, 8 * E:], 0.0)

        # ---------------- embedding ----------------
        xTb = ap_.tile([C, N], BF16, tag="xTb")
        nc.vector.tensor_copy(xTb[:], xT[:])

        st6 = ap_.tile([128, 6 * NQ], F32, tag="st6")
        agg = ap_.tile([128, 2 * NQ], F32, tag="agg")
        rstd = ap_.tile([128, NQ], F32, tag="rstd")
        nmr = ap_.tile([128, NQ], F32, tag="nmr")
        tmpa = ap_.tile([128, NQ], F32, tag="tmpa")

        for j in range(8):
            ep = ps2.tile([128, E], F32, tag="misc")
            nc.tensor.matmul(ep[:], xTb[:, 128 * j:128 * (j + 1)], wemb_sb[:],
                             start=True, stop=False)
            nc.tensor.matmul(ep[:], ones_row[0:1, 0:128], bemb_sb,
                             start=False, stop=True)
            # LN stats for this tile, raw emb parked in h
            nc.vector.bn_stats(st6[:, 6 * j:6 * j + 6], ep[:])
            nc.vector.bn_aggr(agg[:, 2 * j:2 * j + 2], st6[:, 6 * j:6 * j + 6])
            nc.vector.tensor_copy(h[:, E * j:E * (j + 1)], ep[:])
        agg3 = agg[:].rearrange("p (j t) -> p t j", t=2)
        # rstd = exp(-0.5*ln(var+eps)): stays on the natural_log_exp ACT set
        nc.scalar.activation(tmpa[:, 0:8], agg3[:, 1:2, 0:8], AF.Ln,
                             bias=epsb[:, 0:1], scale=1.0)
        nc.scalar.activation(rstd[:, 0:8], tmpa[:, 0:8], AF.Exp,
                             bias=0.0, scale=-0.5)
        for j in range(8):
            sl = slice(E * j, E * (j + 1))
            # (x - m) * rstd, then * g + b (broadcast consts), then gelu
            nc.vector.tensor_scalar(out=h[:, sl], in0=h[:, sl],
                                    scalar1=agg3[:, 0:1, j:j + 1],
                                    scalar2=rstd[:, j:j + 1],
                                    op0=OP.subtract, op1=OP.mult)
            nc.vector.tensor_tensor(out=h[:, sl], in0=h[:, sl],
                                    in1=gemb_sb, op=OP.mult)
            nc.vector.tensor_tensor(out=h[:, sl], in0=h[:, sl],
                                    in1=beemb_sb, op=OP.add)
            nc.scalar.activation(h[:, sl], h[:, sl], AF.Gelu)
        # cls token at sequence position 1024 = tile 8 row 0
        nc.vector.tensor_copy(h[0:1, 8 * E:9 * E], cls_sb)

        # ---------------- helpers ----------------
        def ln_stats(j):
            """per-tile LN stats (DVE) — issued early for pipelining."""
            nc.vector.bn_stats(st6[:, 6 * j:6 * j + 6],
                               h[:, E * j:E * (j + 1)])
            nc.vector.bn_aggr(agg[:, 2 * j:2 * j + 2],
                              st6[:, 6 * j:6 * j + 6])

        def layer_norm(lns, lnb, ytA, ytB, stats_done):
            """token-major LN of h -> feature-major bf16 [128,1025] x2.

            If stats_done, bn_stats/bn_aggr were already issued per-tile.
            """
            y0 = ap_.tile([128, NQ * E], BF16, tag="y0")
            if not stats_done:
                for j in range(NQ):
                    ln_stats(j)
            a3 = agg[:].rearrange("p (j t) -> p t j", t=2)
            nc.scalar.activation(tmpa[:], a3[:, 1:2, :], AF.Ln,
                                 bias=epsb[:, 0:1], scale=1.0)
            nc.scalar.activation(rstd[:], tmpa[:], AF.Exp,
                                 bias=0.0, scale=-0.5)
            for j in range(NQ):
                # y0 = (h - mean) * rstd on DVE (keeps ACT free for the
                # transpose evacuations)
                nc.vector.tensor_scalar(out=y0[:, E * j:E * (j + 1)],
                                        in0=h[:, E * j:E * (j + 1)],
                                        scalar1=a3[:, 0:1, j:j + 1],
                                        scalar2=rstd[:, j:j + 1],
                                        op0=OP.subtract, op1=OP.mult)
            # transpose + affine evac
            for t, yt in ((0, ytA), (1, ytB)):
                for jb in range(3):
                    js = list(range(4 * jb, min(4 * jb + 4, NQ)))
                    tp = ps2.tile([128, 512], BF16, tag="misc")
                    for i, j in enumerate(js):
                        nc.tensor.transpose(
                            tp[:, 128 * i:128 * (i + 1)],
                            y0[:, E * j + 128 * t:E * j + 128 * t + 128],
                            ident[:])
                    w = 128 * len(js) if jb < 2 else 1
                    nc.scalar.activation(yt[:, 512 * jb:512 * jb + w],
                                         tp[:, 0:w], AF.Identity,
                                         bias=lnb[:, t:t + 1],
                                         scale=lns[:, t:t + 1])

        def project_qk(w_sb, bias_fm, ys, qtA, qtB):
            """yT @ W -> feature-major [2][128,1025] bf16 with bias."""
            for m, qt in ((0, qtA), (1, qtB)):
                for c0, cw in ((0, 512), (512, 512), (1024, 1)):
                    pp = ps2.tile([128, 512], F32, tag="misc")
                    for t in range(2):
                        nc.tensor.matmul(
                            pp[:, 0:cw],
                            w_sb[:, E * t + 128 * m:E * t + 128 * m + 128],
                            ys[t][:, c0:c0 + cw],
                            start=(t == 0), stop=(t == 1))
                    nc.scalar.activation(qt[:, c0:c0 + cw], pp[:, 0:cw],
                                         AF.Identity,
                                         bias=bias_fm[:, m:m + 1], scale=1.0)

        # ---------------- transformer layers ----------------
        for l in range(L):
            if l == 0:
                # h += pos; LN1 stats inline (later layers do this per-tile
                # inside the MLP epilogue)
                nc.gpsimd.tensor_tensor(out=h[:], in0=h[:], in1=pos_sb,
                                        op=OP.add)
                for j in range(NQ):
                    ln_stats(j)
            # ---- attention ----
            ytA = ap_.tile([128, S], BF16, tag="ytA")
            ytB = ap_.tile([128, S], BF16, tag="ytB")
            layer_norm(ln1s_sb[l], ln1b_sb[l], ytA, ytB, stats_done=True)

            qtA = ap_.tile([128, S], BF16, tag="qtA")
            qtB = ap_.tile([128, S], BF16, tag="qtB")
            ktA = ap_.tile([128, S], BF16, tag="ktA")
            ktB = ap_.tile([128, S], BF16, tag="ktB")
            project_qk(wq_sb[l], bqf_sb[l], (ytA, ytB), qtA, qtB)
            project_qk(wk_sb[l], bkf_sb[l], (ytA, ytB), ktA, ktB)

            # V token-major with 16.0 augmentation columns
            vsb = ap_.tile([128, NKT * 264], BF16, tag="vsb")
            v4 = vsb[:].rearrange("p (k s e) -> p k s e", k=NKT, s=8)
            nc.vector.memset(v4[:, :, :, 32:33], DIV)
            for kt in range(NKT):
                mw = 128 if kt < 8 else 1
                vp = ps2.tile([128, E], F32, tag="misc")
                for t in range(2):
                    nc.tensor.matmul(
                        vp[0:mw, :],
                        (ytA if t == 0 else ytB)[:, 128 * kt:128 * kt + mw],
                        wv_sb[l][:, E * t:E * (t + 1)],
                        start=(t == 0), stop=(t == 1))
                nc.vector.tensor_copy(
                    v4[0:mw, kt, :, 0:32],
                    vp[0:mw, :].rearrange("p (s d) -> p s d", s=8))
            # replicate the kt=8 V row (+aug) to all partitions so straggler
            # PV matmuls can take their stationary at rows 32/64/96 (HW needs
            # Fmap and Weight to start at the same partition)
            nc.gpsimd.partition_broadcast(vsb[:, 264 * 8:264 * 9],
                                          vsb[0:1, 264 * 8:264 * 9])

            # attention output per slot (normalized O^T, bf16)
            otp = []
            for s in range(8):
                ot_s = ap_.tile([97, S], BF16, tag=f"otp{s}")
                otp.append(ot_s)

            def slot_rows(s):
                qt = qtA if s < 4 else qtB
                kt_t = ktA if s < 4 else ktB
                rp = 32 * (s % 4)
                return qt[rp:rp + 32, :], kt_t[rp:rp + 32, :], rp

            stot = psM.tile([128, 96], F32, tag="stot")
            nc.vector.memset(stot[:], 0.0)
            pairs = [(2 * p, 2 * p + 1) for p in range(4)]
            for sA, sB in pairs:
                pv = psM.tile([128, 1024], F32, tag="pv")
                group = []
                for s in (sA, sB):
                    qr, kr, rp = slot_rows(s)
                    ob = 0 if s == sA else 64
                    group.append((s, qr, kr, rp, ob))
                for kt in range(8):
                    for c0 in (0, 512):
                        engs = []
                        for s, qr, kr, rp, ob in group:
                            eps_t = psE.tile([128, 512], F32, tag="eng")
                            nc.tensor.matmul(
                                eps_t[:],
                                kr[:, 128 * kt:128 * kt + 128],
                                qr[:, c0:c0 + 512],
                                start=True, stop=True,
                                tile_position=(rp, 0))
                            engs.append(eps_t)
                        pts_c = []
                        for (s, qr, kr, rp, ob), eps_t in zip(group, engs):
                            ptt = ptp.tile([128, 512], BF16, tag="pt")
                            # slot-parity engine split: odd slots take DVE for
                            # kts in DVEK so both PV inputs arrive together
                            if s % 2 == 1 and kt in DVEK:
                                etmp = dnp.tile([128, 512], F32, tag="etmp")
                                nc.vector._custom_dve(
                                    EXPP, out=etmp[:], in0=eps_t[:],
                                    s0=EXPA3, s1=EXPA2, imm2=EXPA1)
                                nc.vector._custom_dve(SQ16, out=ptt[:],
                                                      in0=etmp[:])
                            else:
                                nc.scalar.activation(ptt[:], eps_t[:], AF.Exp)
                            pts_c.append(ptt)
                        for (s, qr, kr, rp, ob), ptt in zip(group, pts_c):
                            nc.tensor.matmul(
                                pv[ob:ob + 33, c0:c0 + 512],
                                vsb[:, 264 * kt + 33 * s:264 * kt + 33 * s + 33],
                                ptt[:],
                                start=(kt == 0), stop=False,
                                skip_group_check=True,
                                tile_position=(0, ob))
                # straggler key kt=8: four [1,512] energies packed at rows
                # 0/32/64/96 of one PSUM tile, one exp, four PV accumulates
                eps8 = psE.tile([128, 512], F32, tag="eng")
                for ci, c0 in enumerate((0, 512)):
                    for si, (s, qr, kr, rp, ob) in enumerate(group):
                        r = 64 * si + 32 * ci
                        nc.tensor.matmul(
                            eps8[r:r + 1, :],
                            kr[:, 1024:1025], qr[:, c0:c0 + 512],
                            start=True, stop=True,
                            tile_position=(rp, r))
                ptt8 = ptp.tile([128, 512], BF16, tag="pt")
                nc.scalar.activation(ptt8[0:97, :], eps8[0:97, :], AF.Exp)
                for ci, c0 in enumerate((0, 512)):
                    for si, (s, qr, kr, rp, ob) in enumerate(group):
                        r = 64 * si + 32 * ci
                        nc.tensor.matmul(
                            pv[ob:ob + 33, c0:c0 + 512],
                            vsb[r:r + 1,
                                264 * 8 + 33 * s:264 * 8 + 33 * s + 33],
                            ptt8[r:r + 1, :],
                            start=False, stop=True,
                            skip_group_check=True,
                            tile_position=(r, ob))
                # straggler-query energies + softmax + PV per slot
                for s, qr, kr, rp, ob in group:
                    for kt in range(NKT):
                        mw = 128 if kt < 8 else 1
                        nc.tensor.matmul(
                            stot[0:mw, 9 * s + kt:9 * s + kt + 1],
                            kr[:, 128 * kt:128 * kt + mw],
                            qr[:, 1024:1025],
                            start=True, stop=True,
                            tile_position=(rp, 0))
                    ptsl = ptp.tile([128, NKT], BF16, tag="pts")
                    nc.scalar.activation(ptsl[:], stot[:, 9 * s:9 * s + NKT],
                                         AF.Exp)
                    for kt in range(NKT):
                        mw = 128 if kt < 8 else 1
                        nc.tensor.matmul(
                            stot[ob:ob + 33, 80 + s:81 + s],
                            vsb[0:mw, 264 * kt + 33 * s:
                                264 * kt + 33 * s + 33],
                            ptsl[0:mw, kt:kt + 1],
                            start=(kt == 0), stop=(kt == 8),
                            skip_group_check=True,
                            tile_position=(0, ob))
                # Evacuate the whole PV block (rows 0..96) raw with one wide
                # copy: frees the pv PSUM accumulator early so the next
                # pair's PV can start; the normalization chain below then
                # reads SBUF only.
                pvr = dnp.tile([97, S], F32, tag="pvr")
                nc.vector.tensor_copy(pvr[0:33, 0:1024], pv[0:33, :])
                nc.vector.tensor_copy(pvr[64:97, 0:1024], pv[64:97, :])
                for s, qr_, kr_, rp_, ob in group:
                    nc.vector.tensor_copy(pvr[ob:ob + 33, 1024:1025],
                                          stot[ob:ob + 33, 80 + s:81 + s])
                for s, qr_, kr_, rp_, ob in group:
                    dr = ob + 32
                    dnQ = dnp.tile([1, S], F32, tag="dnQ")
                    nc.gpsimd.dma_start(dnQ[0:1, :], pvr[dr:dr + 1, :])
                    rcp = dnp.tile([1, S], F32, tag="rcp")
                    nc.vector.reciprocal_approx_fast(rcp[:], dnQ[:])
                    # broadcast across all 96 rows (dst base partition 0);
                    # the SBUF-SBUF multiply below then uses matching row
                    # slices so base partitions agree
                    bc = bcp.tile([96, S], F32, tag="bc")
                    nc.gpsimd.partition_broadcast(bc[:], rcp[:])
                    nc.vector.tensor_tensor(out=otp[s][ob:ob + 32, :],
                                            in0=pvr[ob:ob + 32, :],
                                            in1=bc[ob:ob + 32, :], op=OP.mult)

            # Wo projection + residual (+ bo + bv@Wo row); per-tile LN2
            # stats issued right after each tile's residual add.
            for qt_i in range(NQ):
                mw = 128 if qt_i < 8 else 1
                wp = ps2.tile([128, E], F32, tag="misc")
                wp2 = psE.tile([128, E], F32, tag="eng")
                for s in range(0, 8, 2):
                    nc.tensor.matmul(
                        wp[0:mw, :],
                        otp[s][0:32, 128 * qt_i:128 * qt_i + mw],
                        wo_sb[l][0:32, E * s:E * (s + 1)],
                        start=(s == 0), stop=False,
                        tile_position=(0, 0))
                    nc.tensor.matmul(
                        wp2[0:mw, :],
                        otp[s + 1][64:96, 128 * qt_i:128 * qt_i + mw],
                        wo_sb[l][64:96, E * (s + 1):E * (s + 2)],
                        start=(s == 0), stop=(s == 6),
                        tile_position=(64, 0))
                nc.tensor.matmul(wp[0:mw, :], ones_row[0:1, 0:mw],
                                 wob_sb[l], start=False, stop=True,
                                 tile_position=(0, 0))
                sl = slice(E * qt_i, E * (qt_i + 1))
                nc.vector.tensor_tensor(out=h[0:mw, sl], in0=h[0:mw, sl],
                                        in1=wp[0:mw, :], op=OP.add)
                nc.vector.tensor_tensor(out=h[0:mw, sl], in0=h[0:mw, sl],
                                        in1=wp2[0:mw, :], op=OP.add)
                ln_stats(qt_i)

            # ---- MLP ----
            ytA = ap_.tile([128, S], BF16, tag="ytA")
            ytB = ap_.tile([128, S], BF16, tag="ytB")
            layer_norm(ln2s_sb[l], ln2b_sb[l], ytA, ytB, stats_done=True)

            # W1 chunk-major so W2 tiles start while later chunks project
            gt = ap_.tile([128, 8 * S], BF16, tag="gt")
            for c0, cw in ((0, 512), (512, 512), (1024, 1)):
                for f in range(8):
                    gp = ps2.tile([128, 512], F32, tag="misc")
                    for t in range(2):
                        nc.tensor.matmul(
                            gp[:, 0:cw],
                            w1_sb[l][:, FF * t + 128 * f:FF * t + 128 * f + 128],
                            (ytA if t == 0 else ytB)[:, c0:c0 + cw],
                            start=(t == 0), stop=(t == 1))
                    nc.scalar.activation(gt[:, S * f + c0:S * f + c0 + cw],
                                         gp[:, 0:cw], AF.Gelu,
                                         bias=b1f_sb[l][:, f:f + 1], scale=1.0)
            for qt_i in range(NQ):
                mw = 128 if qt_i < 8 else 1
                wp = ps2.tile([128, E], F32, tag="misc")
                for f in range(8):
                    nc.tensor.matmul(wp[0:mw, :],
                                     gt[:, S * f + 128 * qt_i:
                                        S * f + 128 * qt_i + mw],
                                     w2_sb[l][:, E * f:E * (f + 1)],
                                     start=(f == 0), stop=False)
                nc.tensor.matmul(wp[0:mw, :], ones_row[0:1, 0:mw],
                                 b2r_sb[l], start=False, stop=True)
                sl = slice(E * qt_i, E * (qt_i + 1))
                nc.vector.tensor_tensor(out=h[0:mw, sl], in0=h[0:mw, sl],
                                        in1=wp[0:mw, :], op=OP.add)
                if l < L - 1:
                    # next layer's pos-add + LN1 stats, per-tile (gpsimd add
                    # keeps DVE free; stats follow on DVE)
                    nc.gpsimd.tensor_tensor(out=h[0:mw, sl], in0=h[0:mw, sl],
                                            in1=pos_sb[0:mw, sl], op=OP.add)
                    ln_stats(qt_i)

        # ---------------- classifier ----------------
        hbf = ap_.tile([128, NQ * E], BF16, tag="hbf")
        nc.vector.tensor_copy(hbf[:], h[:])
        mp = ps2.tile([1, E], F32, tag="misc")
        for j in range(8):
            nc.tensor.matmul(mp[:], ones_col[:], hbf[:, E * j:E * (j + 1)],
                             start=(j == 0), stop=False)
        nc.tensor.matmul(mp[:], ones_col[0:1, :], hbf[0:1, 8 * E:9 * E],
                         start=False, stop=True)
        pbf = dnp.tile([1, E], BF16, tag="pbf")
        nc.scalar.activation(pbf[:], mp[:], AF.Identity, bias=0.0,
                             scale=1.0 / S)
        # p @ Wc1 + bc1
        ptp_ps = ps2.tile([128, 512], BF16, tag="misc")
        pT = dnp.tile([128, 2], BF16, tag="pT")
        for t in range(2):
            nc.tensor.transpose(ptp_ps[:, 128 * t:128 * t + 1],
                                pbf[0:1, 128 * t:128 * (t + 1)],
                                ident[0:1, 0:1])
            nc.vector.tensor_copy(pT[:, t:t + 1],
                                  ptp_ps[:, 128 * t:128 * t + 1])
        c1p = ps2.tile([1, E], F32, tag="misc")
        for t in range(2):
            nc.tensor.matmul(c1p[:], pT[:, t:t + 1],
                             wc1_sb[:, E * t:E * (t + 1)],
                             start=(t == 0), stop=False)
        nc.tensor.matmul(c1p[:], ones_row[0:1, 0:1], bc1_sb,
                         start=False, stop=True)
        # LN over the [1, E] row
        p2 = dnp.tile([1, E], F32, tag="p2")
        nc.vector.tensor_copy(p2[:], c1p[:])
        st1 = dnp.tile([1, 6], F32, tag="st1")
        ag1 = dnp.tile([1, 2], F32, tag="ag1")
        nc.vector.bn_stats(st1[:], p2[:])
        nc.vector.bn_aggr(ag1[:], st1[:])
        r1 = dnp.tile([1, 2], F32, tag="r1")
        nc.scalar.activation(r1[:, 0:1], ag1[:, 1:2], AF.Ln,
                             bias=epsb[0:1, 0:1], scale=1.0)
        nc.scalar.activation(r1[:, 0:1], r1[:, 0:1], AF.Exp,
                             bias=0.0, scale=-0.5)
        nc.vector.tensor_scalar(out=p2[:], in0=p2[:], scalar1=ag1[:, 0:1],
                                scalar2=r1[:, 0:1], op0=OP.subtract,
                                op1=OP.mult)
        nc.vector.tensor_tensor(out=p2[:], in0=p2[:], in1=lncg_sb,
                                op=OP.mult)
        nc.vector.tensor_tensor(out=p2[:], in0=p2[:], in1=lncb_sb,
                                op=OP.add)
        p2b = dnp.tile([1, E], BF16, tag="p2b")
        nc.vector.tensor_copy(p2b[:], p2[:])
        p2T = dnp.tile([128, 2], BF16, tag="p2T")
        for t in range(2):
            tp2 = ps2.tile([128, 512], BF16, tag="misc")
            nc.tensor.transpose(tp2[:, 0:1], p2b[0:1, 128 * t:128 * (t + 1)],
                                ident[0:1, 0:1])
            nc.vector.tensor_copy(p2T[:, t:t + 1], tp2[:, 0:1])
        op_ = ps2.tile([1, NCLS], F32, tag="misc")
        for t in range(2):
            nc.tensor.matmul(op_[:], p2T[:, t:t + 1],
                             wc2_sb[:, NCLS * t:NCLS * (t + 1)],
                             start=(t == 0), stop=False)
        nc.tensor.matmul(op_[:], ones_row[0:1, 0:1], bc2_sb,
                         start=False, stop=True)
        osb = dnp.tile([1, NCLS], F32, tag="osb")
        nc.vector.tensor_copy(osb[:], op_[:])
        nc.sync.dma_start(out_d[:], osb[:])

        for _p in (ps2, psE, psM, dnp, bcp, ptp, ap_, cp):
            _p.release()

    nc.compile()
    return nc


def _prep_shared(inputs):
    """Host-side weight preparation (shared across cores)."""
    bf16 = ml_dtypes.bfloat16
    f32 = np.float32
    g = {k: np.asarray(v, dtype=f32) for k, v in inputs.items()}
    d = {}

    # sequence order [tokens 0..1023, cls] -> permute pos accordingly
    pos_perm = np.concatenate([g["pos"][1:], g["pos"][0:1]], axis=0)
    pos_tm = np.zeros((128, NQ * E), f32)
    for j in range(NQ):
        n = 128 if j < 8 else 1
        pos_tm[0:n, E * j:E * (j + 1)] = pos_perm[128 * j:128 * j + n]

    d["wemb"] = g["W_emb"].astype(bf16)

    perm = np.concatenate([np.arange(32) + 32 * SIG[s] for s in range(8)])

    def pack_k(w):  # [256, X] -> [128, 2X]
        return np.concatenate([w[0:128], w[128:256]], axis=1)

    # rowsb: packed bf16 bias rows on partition 0
    rowsb = np.zeros((1, 2565), f32)
    rowsb[0, RBEMB:RBEMB + E] = g["b_emb"]
    for l in range(L):
        rowsb[0, RWOB + E * l:RWOB + E * (l + 1)] = \
            g["bo"][l] + g["bv"][l] @ g["Wo"][l]
        rowsb[0, RB2R + E * l:RB2R + E * (l + 1)] = g["b2"][l]
    rowsb[0, RBC1:RBC1 + E] = g["bc1"]
    rowsb[0, RBC2:RBC2 + NCLS] = g["bc2"]
    d["rowsb"] = rowsb.astype(bf16)

    rowsf = np.zeros((1, 3 * E), f32)
    rowsf[0, 0:E] = g["cls_token"].reshape(E)
    rowsf[0, E:2 * E] = g["lnc_g"]
    rowsf[0, 2 * E:3 * E] = g["lnc_b"]
    d["rowsf"] = rowsf

    smallf = np.zeros((128, 2896), f32)
    smallf[:, OPOS:OPOS + NQ * E] = pos_tm
    smallf[:, OGEMB:OGEMB + E] = g["g_emb"][None]
    smallf[:, OBEEMB:OBEEMB + E] = g["be_emb"][None]
    for l in range(L):
        o = OSM + 20 * l
        smallf[:, o + 0:o + 2] = pack_k(g["ln1_g"][l].reshape(E, 1))
        smallf[:, o + 2:o + 4] = pack_k(g["ln1_b"][l].reshape(E, 1))
        smallf[:, o + 4:o + 6] = pack_k(g["ln2_g"][l].reshape(E, 1))
        smallf[:, o + 6:o + 8] = pack_k(g["ln2_b"][l].reshape(E, 1))
        smallf[:, o + 8:o + 10] = pack_k(g["bq"][l][perm].reshape(E, 1))
        smallf[:, o + 10:o + 12] = pack_k(g["bk"][l][perm].reshape(E, 1))
        smallf[:, o + 12:o + 20] = np.stack(
            [g["b1"][l][128 * t:128 * (t + 1)] for t in range(8)], axis=1)
    d["smallf"] = smallf

    for l in range(L):
        big = np.zeros((128, 7680), f32)
        big[:, OWQ:OWQ + 2 * E] = pack_k(g["Wq"][l][:, perm])
        big[:, OWK:OWK + 2 * E] = pack_k(g["Wk"][l][:, perm])
        big[:, OWV:OWV + 2 * E] = pack_k(g["Wv"][l][:, perm])
        # wo: slot s cols E*s hold Wo rows of head SIG[s], at partition
        # rows 0..31 for even s and 64..95 for odd s.
        for s in range(8):
            r0 = 0 if s % 2 == 0 else 64
            big[r0:r0 + 32, OWO + E * s:OWO + E * (s + 1)] = \
                g["Wo"][l][32 * SIG[s]:32 * SIG[s] + 32]
        big[:, OW1:OW1 + 2 * FF] = pack_k(g["W1"][l])
        big[:, OW2:OW2 + 8 * E] = np.concatenate(
            [g["W2"][l][128 * t:128 * (t + 1)] for t in range(8)], axis=1)
        d[f"bigw{l}"] = big.astype(bf16)

    wcp = np.zeros((128, 2 * E + 2 * NCLS), f32)
    wcp[:, 0:2 * E] = pack_k(g["Wc1"])
    wcp[:, 2 * E:] = pack_k(g["Wc2"])
    d["wc"] = wcp.astype(bf16)
    return d


def kernel(**inputs):
    sys.path.insert(0, TRN_REPO)
    from concourse.bass_utils import run_bass_kernel_spmd

    if "nc" not in _CACHE:
        _CACHE["nc"] = _build()
    nc = _CACHE["nc"]

    shared = _prep_shared(inputs)
    x = np.asarray(inputs["x"], dtype=np.float32)
    in_maps = [dict(shared, x=np.ascontiguousarray(x[c])) for c in range(B)]
    res = run_bass_kernel_spmd(nc, in_maps, list(range(B)))
    out = np.stack([np.asarray(res.results[c]["out"]).reshape(NCLS)
                    for c in range(B)])
    return out.astype(np.float32)


# revision 32
# speedup vs baseline: 1.5589x; 1.1820x over previous
"""ECGformer forward pass on 8 TRN2 NeuronCores, data-parallel over batch.

Layout strategy per core (1 batch element):
- Residual stream h: token-major fp32, [128, 9*256].  Sequence order is
  [tokens 0..1023, cls]: the cls token lives at tile 8 row 0 (pos table is
  permuted to match on host).  Full softmax + mean-pool are permutation
  invariant, so this matches the reference's [cls, tokens] order exactly
  while letting the embedding write h tiles 0..7 in place (no shift DMAs).
- LayerNorm in token-major (per-partition stats); rstd = exp(-0.5*ln(v+eps))
  on ACT so the whole attention phase stays on the natural_log_exp table set
  (no sqrt-set thrash); affine applied during the PE-transpose evacuation.
- Attention in [k, q] feature-major: energy^T = K^T-slice.T @ Q^T via K=32
  row-tiled matmuls; softmax denominator comes free from a 16.0-valued
  augmentation column in V (also folds the post-softmax /sqrt(E) division);
  O^T accumulated per head with M=33 matmuls.  Energies/exp/PV run as a
  software pipeline with two-unit lookahead ([128,1024] PSUM tiles holding
  both slots of a pair); straggler key (kt=8) and straggler query (q=1024)
  are batched per layer into memset PSUM tiles with one exp each.
- Weights arrive in a handful of large packed DMAs (one [128,7680] tensor
  per layer + small packed row/col tensors), issued after the x transpose so
  compute starts immediately.
- All matmul operands bf16 (fp32 PSUM accumulation); weights permuted so
  attention head h lives in "slot" s with 32-aligned rows.
"""

import os
import sys

import numpy as np

try:
    import ml_dtypes
except ImportError:  # pragma: no cover
    ml_dtypes = None

TRN_REPO = "/opt/trn_rl_repo"

B, N, C = 8, 1024, 12
E, H, HD, L, FF, NCLS = 256, 8, 32, 4, 1024, 5
S = N + 1          # 1025 tokens
NQ = 9             # token tiles (8 full + 1 single-row)
NKT = 9            # k tiles in attention
EPS = 1e-5
SIG = [0, 4, 1, 5, 2, 6, 3, 7]   # slot s holds head SIG[s]
DIV = 16.0         # sqrt(E); folded into the V augmentation column

# packed big-weight column offsets (bf16, [128, 7680] per layer)
OWQ, OWK, OWV, OWO, OW1, OW2 = 0, 512, 1024, 1536, 3584, 5632
# packed small-f32 column offsets ([128, 2896])
OPOS, OGEMB, OBEEMB, OSM = 0, 2304, 2560, 2816   # per-layer smalls at OSM+20*l
# packed bias-row offsets (bf16, [1, 2565])
RBEMB, RWOB, RB2R, RBC1, RBC2 = 0, 256, 1280, 2304, 2560

_CACHE = {}
DVEK = {int(v) for v in os.environ.get("DVEK", "5,10,15").split(",") if v != ""}


EXPA1, EXPA2, EXPA3 = 0.0625147850914497, 0.0019667051147439383, 4.0039417979884645e-05


def _register_exp_ops():
    import re
    from concourse import dve_ops as D
    from concourse.dve_spec import Spec, Src0, C0, C1, C2, One, sq
    if any(op.name == "ANT_EXPP" for op in D.OPS):
        return
    expp = D.DveOp(
        name="ANT_EXPP",
        spec=Spec(
            body=((C0 * Src0 + C1) * Src0 + C2) * Src0 + One,
            reference=lambda in0, in1=None, s0=0.0, s1=0.0, imm2=0.0:
                ((s0 * in0 + s1) * in0 + imm2) * in0 + 1.0,
        ),
        subdim=False, uops_sha={})
    sq16 = D.DveOp(
        name="ANT_SQ16",
        spec=Spec(body=sq(sq(sq(sq(Src0)))),
                  reference=lambda in0, *a: (((in0 ** 2) ** 2) ** 2) ** 2),
        subdim=False, uops_sha={})
    for op in (expp, sq16):
        D.OPS.append(op)
        D.CUSTOM_DVE_SPECS[op.name] = op.spec
        D._SUB_OPCODE_FOR_NAME[op.name] = D._CUSTOM_DVE_ROW_BASE + len(D.OPS) - 1
        assert D._SUB_OPCODE_FOR_NAME[op.name] < 0x20
        for ver in ("v3", "v4"):
            try:
                op.compile(ver)
            except ValueError as e:
                m = re.search(r"\(" + ver + r": ([0-9a-f]+)", str(e))
                op.uops_sha[ver] = m.group(1)
        op.compile("v3")
    return expp, sq16


def _build():
    sys.path.insert(0, TRN_REPO)
    import concourse.tile as tile
    from concourse import mybir, bacc
    from concourse.masks import make_identity
    from concourse import dve_ops as D
    _register_exp_ops()
    EXPP = next(op for op in D.OPS if op.name == "ANT_EXPP")
    SQ16 = next(op for op in D.OPS if op.name == "ANT_SQ16")

    F32 = mybir.dt.float32
    BF16 = mybir.dt.bfloat16
    AF = mybir.ActivationFunctionType
    OP = mybir.AluOpType

    nc = bacc.Bacc("TRN2", target_bir_lowering=False, debug=False, num_devices=8)

    # ---------------- DRAM parameters ----------------
    def din(name, shape, dt=F32):
        return nc.dram_tensor(name, shape, dt, kind="ExternalInput").ap()

    x_d = din("x", [N, C])
    wemb_d = din("wemb", [C, E], BF16)
    rowsb_d = din("rowsb", [1, 2565], BF16)
    rowsf_d = din("rowsf", [1, 3 * E])
    smallf_d = din("smallf", [128, 2896])
    bigw_d = [din(f"bigw{l}", [128, 7680], BF16) for l in range(L)]
    wc_d = din("wc", [128, 2 * E + 2 * NCLS], BF16)

    out_d = nc.dram_tensor("out", [1, NCLS], F32, kind="ExternalOutput").ap()

    with tile.TileContext(nc) as tc:
        cp = tc.alloc_tile_pool(name="consts", bufs=1)
        ap_ = tc.alloc_tile_pool(name="acts", bufs=1)
        ptp = tc.alloc_tile_pool(name="ptp", bufs=5)
        bcp = tc.alloc_tile_pool(name="bcp", bufs=2)
        dnp = tc.alloc_tile_pool(name="dnp", bufs=2)
        dn1 = tc.alloc_tile_pool(name="dn1", bufs=1)
        psV = tc.alloc_tile_pool(name="psV", bufs=1, space="PSUM")
        psE = tc.alloc_tile_pool(name="psE", bufs=3, space="PSUM")

        # ---------------- input + constants into SBUF (x first) --------
        xnat = ap_.tile([128, 8 * C], F32, tag="xnat")
        nc.sync.dma_start(xnat[:].rearrange("p (j c) -> p j c", c=C),
                          x_d[:].rearrange("(j p) c -> p j c", p=128))

        def load(name, dram, shape, dt=F32):
            t = cp.tile(shape, dt, tag=name)
            nc.sync.dma_start(t[:], dram[:])
            return t

        wemb_sb = load("wemb", wemb_d, [C, E], BF16)
        rowsb = load("rowsb", rowsb_d, [1, 2565], BF16)
        smallf = load("smallf", smallf_d, [128, 2896])
        rowsf = load("rowsf", rowsf_d, [1, 3 * E])
        bigw = [load(f"bigw{l}", bigw_d[l], [128, 7680], BF16) for l in range(L)]
        wc_sb = load("wc", wc_d, [128, 2 * E + 2 * NCLS], BF16)

        # views into the packed tensors
        bemb_sb = rowsb[0:1, RBEMB:RBEMB + E]
        wob_sb = [rowsb[0:1, RWOB + E * l:RWOB + E * (l + 1)] for l in range(L)]
        b2r_sb = [rowsb[0:1, RB2R + E * l:RB2R + E * (l + 1)] for l in range(L)]
        bc1_sb = rowsb[0:1, RBC1:RBC1 + E]
        bc2_sb = rowsb[0:1, RBC2:RBC2 + NCLS]
        cls_sb = rowsf[0:1, 0:E]
        lncg_sb = rowsf[0:1, E:2 * E]
        lncb_sb = rowsf[0:1, 2 * E:3 * E]
        pos_sb = smallf[:, OPOS:OPOS + NQ * E]
        gemb_sb = smallf[:, OGEMB:OGEMB + E]
        beemb_sb = smallf[:, OBEEMB:OBEEMB + E]
        sm = [smallf[:, OSM + 20 * l:OSM + 20 * (l + 1)] for l in range(L)]
        ln1s_sb = [sm[l][:, 0:2] for l in range(L)]
        ln1b_sb = [sm[l][:, 2:4] for l in range(L)]
        ln2s_sb = [sm[l][:, 4:6] for l in range(L)]
        ln2b_sb = [sm[l][:, 6:8] for l in range(L)]
        bqf_sb = [sm[l][:, 8:10] for l in range(L)]
        bkf_sb = [sm[l][:, 10:12] for l in range(L)]
        b1f_sb = [sm[l][:, 12:20] for l in range(L)]
        wq_sb = [bigw[l][:, OWQ:OWQ + 2 * E] for l in range(L)]
        wk_sb = [bigw[l][:, OWK:OWK + 2 * E] for l in range(L)]
        wv_sb = [bigw[l][:, OWV:OWV + 2 * E] for l in range(L)]
        wo_sb = [bigw[l][:, OWO:OWO + 8 * E] for l in range(L)]
        w1_sb = [bigw[l][:, OW1:OW1 + 2 * FF] for l in range(L)]
        w2_sb = [bigw[l][:, OW2:OW2 + 8 * E] for l in range(L)]
        wc1_sb = wc_sb[:, 0:2 * E]
        wc2_sb = wc_sb[:, 2 * E:2 * E + 2 * NCLS]

        ident = cp.tile([128, 128], BF16, tag="ident")
        make_identity(nc, ident[:])
        epsb = cp.tile([128, 1], F32, tag="epsb")
        nc.vector.memset(epsb[:], EPS)
        ones_row = cp.tile([1, 128], BF16, tag="ones_row")
        nc.vector.memset(ones_row[:], 1.0)
        ones_col = cp.tile([128, 1], BF16, tag="ones_col")
        nc.vector.memset(ones_col[:], 1.0)

        # residual stream
        h = cp.tile([128, NQ * E], F32, tag="h")
        nc.vector.memset(h[:, 8 * E:], 0.0)

        # ---------------- embedding ----------------
        # transpose x tiles on the PE: [128,12] -> [12,128] each (bf16)
        xnb = ap_.tile([128, 8 * C], BF16, tag="xnb")
        nc.vector.tensor_copy(xnb[:], xnat[:])
        xTb = ap_.tile([C, N], BF16, tag="xTb")
        for j in range(8):
            xp = psE.tile([128, 512], BF16, tag="eng")
            nc.tensor.transpose(xp[0:C, 0:128], xnb[:, C * j:C * (j + 1)],
                                ident[:])
            nc.vector.tensor_copy(xTb[:, 128 * j:128 * (j + 1)],
                                  xp[0:C, 0:128])

        st6 = ap_.tile([128, 6 * NQ], F32, tag="st6")
        agg = ap_.tile([128, 2 * NQ], F32, tag="agg")
        rstd = ap_.tile([128, NQ], F32, tag="rstd")
        tmpa = ap_.tile([128, NQ], F32, tag="tmpa")

        for j in range(8):
            ep = psE.tile([128, E], F32, tag="eng")
            nc.tensor.matmul(ep[:], xTb[:, 128 * j:128 * (j + 1)], wemb_sb[:],
                             start=True, stop=False)
            nc.tensor.matmul(ep[:], ones_row[0:1, 0:128], bemb_sb,
                             start=False, stop=True)
            # LN stats for this tile, raw emb parked in h
            nc.vector.bn_stats(st6[:, 6 * j:6 * j + 6], ep[:])
            nc.vector.bn_aggr(agg[:, 2 * j:2 * j + 2], st6[:, 6 * j:6 * j + 6])
            nc.vector.tensor_copy(h[:, E * j:E * (j + 1)], ep[:])
        agg3 = agg[:].rearrange("p (j t) -> p t j", t=2)
        # rstd = exp(-0.5*ln(var+eps)): stays on the natural_log_exp ACT set
        nc.scalar.activation(tmpa[:, 0:8], agg3[:, 1:2, 0:8], AF.Ln,
                             bias=epsb[:, 0:1], scale=1.0)
        nc.scalar.activation(rstd[:, 0:8], tmpa[:, 0:8], AF.Exp,
                             bias=0.0, scale=-0.5)
        for j in range(8):
            sl = slice(E * j, E * (j + 1))
            # (x - m) * rstd, then * g + b (broadcast consts), then gelu
            nc.vector.tensor_scalar(out=h[:, sl], in0=h[:, sl],
                                    scalar1=agg3[:, 0:1, j:j + 1],
                                    scalar2=rstd[:, j:j + 1],
                                    op0=OP.subtract, op1=OP.mult)
            nc.vector.tensor_tensor(out=h[:, sl], in0=h[:, sl],
                                    in1=gemb_sb, op=OP.mult)
            nc.vector.tensor_tensor(out=h[:, sl], in0=h[:, sl],
                                    in1=beemb_sb, op=OP.add)
            nc.scalar.activation(h[:, sl], h[:, sl], AF.Gelu)
        # cls token at sequence position 1024 = tile 8 row 0
        nc.vector.tensor_copy(h[0:1, 8 * E:9 * E], cls_sb)

        # ---------------- helpers ----------------
        def ln_stats(j):
            """per-tile LN stats (DVE) — issued early for pipelining."""
            nc.vector.bn_stats(st6[:, 6 * j:6 * j + 6],
                               h[:, E * j:E * (j + 1)])
            nc.vector.bn_aggr(agg[:, 2 * j:2 * j + 2],
                              st6[:, 6 * j:6 * j + 6])

        def layer_norm(lns, lnb, ytA, ytB, stats_done):
            """token-major LN of h -> feature-major bf16 [128,1025] x2.

            If stats_done, bn_stats/bn_aggr were already issued per-tile.
            """
            y0 = ap_.tile([128, NQ * E], BF16, tag="y0")
            if not stats_done:
                for j in range(NQ):
                    ln_stats(j)
            a3 = agg[:].rearrange("p (j t) -> p t j", t=2)
            nc.scalar.activation(tmpa[:], a3[:, 1:2, :], AF.Ln,
                                 bias=epsb[:, 0:1], scale=1.0)
            nc.scalar.activation(rstd[:], tmpa[:], AF.Exp,
                                 bias=0.0, scale=-0.5)
            for j in range(NQ):
                # y0 = (h - mean) * rstd on DVE (keeps ACT free for the
                # transpose evacuations)
                nc.vector.tensor_scalar(out=y0[:, E * j:E * (j + 1)],
                                        in0=h[:, E * j:E * (j + 1)],
                                        scalar1=a3[:, 0:1, j:j + 1],
                                        scalar2=rstd[:, j:j + 1],
                                        op0=OP.subtract, op1=OP.mult)
            # transpose + affine evac
            for t, yt in ((0, ytA), (1, ytB)):
                for jb in range(3):
                    js = list(range(4 * jb, min(4 * jb + 4, NQ)))
                    tp = psE.tile([128, 512], BF16, tag="eng")
                    for i, j in enumerate(js):
                        nc.tensor.transpose(
                            tp[:, 128 * i:128 * (i + 1)],
                            y0[:, E * j + 128 * t:E * j + 128 * t + 128],
                            ident[:])
                    w = 128 * len(js) if jb < 2 else 1
                    nc.scalar.activation(yt[:, 512 * jb:512 * jb + w],
                                         tp[:, 0:w], AF.Identity,
                                         bias=lnb[:, t:t + 1],
                                         scale=lns[:, t:t + 1])

        def project_qk(w_sb, bias_fm, ys, qtA, qtB):
            """yT @ W -> feature-major [2][128,1025] bf16 with bias."""
            for m, qt in ((0, qtA), (1, qtB)):
                for c0, cw in ((0, 512), (512, 512), (1024, 1)):
                    pp = psE.tile([128, 512], F32, tag="eng")
                    for t in range(2):
                        nc.tensor.matmul(
                            pp[:, 0:cw],
                            w_sb[:, E * t + 128 * m:E * t + 128 * m + 128],
                            ys[t][:, c0:c0 + cw],
                            start=(t == 0), stop=(t == 1))
                    nc.scalar.activation(qt[:, c0:c0 + cw], pp[:, 0:cw],
                                         AF.Identity,
                                         bias=bias_fm[:, m:m + 1], scale=1.0)

        # ---------------- transformer layers ----------------
        for l in range(L):
            if l == 0:
                # h += pos; LN1 stats inline (later layers do this per-tile
                # inside the MLP epilogue)
                nc.gpsimd.tensor_tensor(out=h[:], in0=h[:], in1=pos_sb,
                                        op=OP.add)
                for j in range(NQ):
                    ln_stats(j)
            # ---- attention ----
            ytA = ap_.tile([128, S], BF16, tag="ytA")
            ytB = ap_.tile([128, S], BF16, tag="ytB")
            layer_norm(ln1s_sb[l], ln1b_sb[l], ytA, ytB, stats_done=True)

            qtA = ap_.tile([128, S], BF16, tag="qtA")
            qtB = ap_.tile([128, S], BF16, tag="qtB")
            ktA = ap_.tile([128, S], BF16, tag="ktA")
            ktB = ap_.tile([128, S], BF16, tag="ktB")
            project_qk(wq_sb[l], bqf_sb[l], (ytA, ytB), qtA, qtB)
            project_qk(wk_sb[l], bkf_sb[l], (ytA, ytB), ktA, ktB)

            # V token-major with 16.0 augmentation columns
            vsb = ap_.tile([128, NKT * 264], BF16, tag="vsb")
            v4 = vsb[:].rearrange("p (k s e) -> p k s e", k=NKT, s=8)
            nc.vector.memset(v4[:, :, :, 32:33], DIV)
            for kt in range(NKT):
                mw = 128 if kt < 8 else 1
                vp = psE.tile([128, E], F32, tag="eng")
                for t in range(2):
                    nc.tensor.matmul(
                        vp[0:mw, :],
                        (ytA if t == 0 else ytB)[:, 128 * kt:128 * kt + mw],
                        wv_sb[l][:, E * t:E * (t + 1)],
                        start=(t == 0), stop=(t == 1))
                nc.vector.tensor_copy(
                    v4[0:mw, kt, :, 0:32],
                    vp[0:mw, :].rearrange("p (s d) -> p s d", s=8))
            # replicate the kt=8 V row (+aug) to all partitions so straggler
            # PV matmuls can take their stationary at rows 32/64/96 (HW needs
            # Fmap and Weight to start at the same partition)
            nc.gpsimd.partition_broadcast(vsb[:, 264 * 8:264 * 9],
                                          vsb[0:1, 264 * 8:264 * 9])

            # attention output per slot (normalized O^T, bf16)
            otp = []
            for s in range(8):
                ot_s = ap_.tile([97, S], BF16, tag=f"otp{s}")
                otp.append(ot_s)

            def slot_rows(s):
                qt = qtA if s < 4 else qtB
                kt_t = ktA if s < 4 else ktB
                rp = 32 * (s % 4)
                return qt[rp:rp + 32, :], kt_t[rp:rp + 32, :], rp

            # straggler key kt=8 for all 8 slots, batched per layer: two
            # memset PSUM tiles (rows 0/32/64/96 x both c0 halves), one exp
            # each; the per-pair PV accumulates read rows of these tiles.
            ptt8L = []
            for g2 in range(2):
                e8 = psE.tile([128, 1024], F32, tag="eng")
                nc.vector.memset(e8[0:97, :], 0.0)
                for sloc in range(4):
                    s8 = 4 * g2 + sloc
                    qr8, kr8, rp8 = slot_rows(s8)
                    r = 32 * sloc
                    for c0 in (0, 512):
                        nc.tensor.matmul(
                            e8[r:r + 1, c0:c0 + 512],
                            kr8[:, 1024:1025], qr8[:, c0:c0 + 512],
                            start=True, stop=True,
                            tile_position=(rp8, r))
                p8t = ptp.tile([128, 1024], BF16, tag="pt8")
                nc.scalar.activation(p8t[0:97, :], e8[0:97, :], AF.Exp)
                ptt8L.append(p8t)

            # straggler query q=1024, batched per layer: two PSUM tiles of
            # 4 slots each; energies 4-way row-packed (one slot per row
            # group), one exp per tile, PV columns 2-way col-packed into
            # cols 512+sloc, then one [97,4] evac per tile.
            stqcol = []
            for g2 in range(2):
                stq = psE.tile([128, 1024], F32, tag="eng")
                nc.vector.memset(stq[:, 0:600], 0.0)
                sl4 = [(4 * g2 + i,) + slot_rows(4 * g2 + i) for i in range(4)]
                for sloc, (s8, qr8, kr8, rp8) in enumerate(sl4):
                    for kt in range(NKT):
                        mw = 128 if kt < 8 else 1
                        nc.tensor.matmul(
                            stq[0:mw, 16 * sloc + kt:16 * sloc + kt + 1],
                            kr8[:, 128 * kt:128 * kt + mw],
                            qr8[:, 1024:1025],
                            start=True, stop=True,
                            tile_position=(rp8, 0))
                ptq = ptp.tile([128, 64], BF16, tag="pts")
                nc.scalar.activation(ptq[:, 0:57], stq[:, 0:57], AF.Exp)
                for sloc, (s8, qr8, kr8, rp8) in enumerate(sl4):
                    ob8 = 0 if s8 % 2 == 0 else 64
                    for kt in range(NKT):
                        mw = 128 if kt < 8 else 1
                        nc.tensor.matmul(
                            stq[ob8:ob8 + 33, 512 + sloc:513 + sloc],
                            vsb[0:mw, 264 * kt + 33 * s8:
                                264 * kt + 33 * s8 + 33],
                            ptq[0:mw, 16 * sloc + kt:16 * sloc + kt + 1],
                            start=(kt == 0), stop=(kt == 8),
                            skip_group_check=True,
                            tile_position=(0, ob8))
                sc = dnp.tile([97, 4], F32, tag=f"stqc{g2}")
                nc.vector.tensor_copy(sc[:], stq[0:97, 512:516])
                stqcol.append(sc)

            pairs = [(2 * p, 2 * p + 1) for p in range(4)]
            for sA, sB in pairs:
                pv = psV.tile([128, 1024], F32, tag="pv")
                group = []
                for s in (sA, sB):
                    qr, kr, rp = slot_rows(s)
                    ob = 0 if s == sA else 64
                    group.append((s, qr, kr, rp, ob))

                # Software-pipelined units: one [128,1024] PSUM tile holds
                # BOTH slots' energies for one (kt, c0); a single 1024-wide
                # exp (ACT or DVE) converts it; the two PV matmuls for unit
                # i-1 are issued after unit i's energies so the exp latency
                # is hidden and the PE never queue-blocks on a fresh exp.
                units = [(kt, c0) for kt in range(8) for c0 in (0, 512)]
                pending = []

                def issue_pv(ptt_u, c0_u, kt_u):
                    for si, (s, qr, kr, rp, ob) in enumerate(group):
                        nc.tensor.matmul(
                            pv[ob:ob + 33, c0_u:c0_u + 512],
                            vsb[:, 264 * kt_u + 33 * s:264 * kt_u + 33 * s + 33],
                            ptt_u[:, 512 * si:512 * si + 512],
                            start=(kt_u == 0), stop=False,
                            skip_group_check=True,
                            tile_position=(0, ob))

                for ui, (kt, c0) in enumerate(units):
                    eng = psE.tile([128, 1024], F32, tag="eng")
                    for si, (s, qr, kr, rp, ob) in enumerate(group):
                        nc.tensor.matmul(
                            eng[:, 512 * si:512 * si + 512],
                            kr[:, 128 * kt:128 * kt + 128],
                            qr[:, c0:c0 + 512],
                            start=True, stop=True,
                            tile_position=(rp, 0))
                    ptt = ptp.tile([128, 1024], BF16, tag="pt")
                    if ui in DVEK:
                        etmp = dn1.tile([128, 1024], F32, tag="etmp")
                        nc.vector._custom_dve(
                            EXPP, out=etmp[:], in0=eng[:],
                            s0=EXPA3, s1=EXPA2, imm2=EXPA1)
                        nc.vector._custom_dve(SQ16, out=ptt[:], in0=etmp[:])
                    else:
                        nc.scalar.activation(ptt[:], eng[:], AF.Exp)
                    pending.append((ptt, c0, kt))
                    # four-unit lookahead: by the time PV(u-4) reaches the
                    # PE queue its exp has had four full units to complete,
                    # covering DVE-exp latency and queueing behind the
                    # previous pair's normalize work
                    if len(pending) > 4:
                        issue_pv(*pending.pop(0))

                while pending:
                    issue_pv(*pending.pop(0))
                p8t = ptt8L[sA // 4]
                for si, (s, qr, kr, rp, ob) in enumerate(group):
                    r = 32 * (s % 4)
                    for c0 in (0, 512):
                        nc.tensor.matmul(
                            pv[ob:ob + 33, c0:c0 + 512],
                            vsb[r:r + 1,
                                264 * 8 + 33 * s:264 * 8 + 33 * s + 33],
                            p8t[r:r + 1, c0:c0 + 512],
                            start=False, stop=True,
                            skip_group_check=True,
                            tile_position=(r, ob))

                # Evacuate the PV block raw (frees the single-buffered pv
                # accumulator so the next pair's PV starts immediately), then
                # the baseline normalize chain: denominator row hopped to
                # partition 0 (gpsimd DMA), reciprocal, partition-broadcast,
                # one multiply per slot.
                pvr = dnp.tile([97, S], F32, tag="pvr")
                nc.vector.tensor_copy(pvr[0:33, 0:1024], pv[0:33, :])
                nc.vector.tensor_copy(pvr[64:97, 0:1024], pv[64:97, :])
                scg = stqcol[sA // 4]
                for si, (s, qr_, kr_, rp_, ob) in enumerate(group):
                    nc.vector.tensor_copy(pvr[ob:ob + 33, 1024:1025],
                                          scg[ob:ob + 33, s % 4:s % 4 + 1])
                for si, (s, qr_, kr_, rp_, ob) in enumerate(group):
                    dr = ob + 32
                    dnQ = dn1.tile([1, S], F32, tag="dnQ")
                    nc.gpsimd.dma_start(dnQ[0:1, :], pvr[dr:dr + 1, :])
                    rcp = dn1.tile([1, S], F32, tag="rcp")
                    nc.vector.reciprocal_approx_fast(rcp[:], dnQ[:])
                    bc = bcp.tile([96, S], F32, tag="bc")
                    nc.gpsimd.partition_broadcast(bc[:], rcp[:])
                    nc.vector.tensor_tensor(out=otp[s][ob:ob + 32, :],
                                            in0=pvr[ob:ob + 32, :],
                                            in1=bc[ob:ob + 32, :], op=OP.mult)

            # Wo projection + residual (+ bo + bv@Wo row); per-tile LN2
            # stats issued right after each tile's residual add.
            for qt_i in range(NQ):
                mw = 128 if qt_i < 8 else 1
                wp = psE.tile([128, E], F32, tag="eng")
                wp2 = psE.tile([128, E], F32, tag="eng")
                for s in range(0, 8, 2):
                    nc.tensor.matmul(
                        wp[0:mw, :],
                        otp[s][0:32, 128 * qt_i:128 * qt_i + mw],
                        wo_sb[l][0:32, E * s:E * (s + 1)],
                        start=(s == 0), stop=False,
                        tile_position=(0, 0))
                    nc.tensor.matmul(
                        wp2[0:mw, :],
                        otp[s + 1][64:96, 128 * qt_i:128 * qt_i + mw],
                        wo_sb[l][64:96, E * (s + 1):E * (s + 2)],
                        start=(s == 0), stop=(s == 6),
                        tile_position=(64, 0))
                nc.tensor.matmul(wp[0:mw, :], ones_row[0:1, 0:mw],
                                 wob_sb[l], start=False, stop=True,
                                 tile_position=(0, 0))
                sl = slice(E * qt_i, E * (qt_i + 1))
                nc.vector.tensor_tensor(out=h[0:mw, sl], in0=h[0:mw, sl],
                                        in1=wp[0:mw, :], op=OP.add)
                nc.vector.tensor_tensor(out=h[0:mw, sl], in0=h[0:mw, sl],
                                        in1=wp2[0:mw, :], op=OP.add)
                ln_stats(qt_i)

            # ---- MLP ----
            ytA = ap_.tile([128, S], BF16, tag="ytA")
            ytB = ap_.tile([128, S], BF16, tag="ytB")
            layer_norm(ln2s_sb[l], ln2b_sb[l], ytA, ytB, stats_done=True)

            # W1 chunk-major so W2 tiles start while later chunks project
            gt = ap_.tile([128, 8 * S], BF16, tag="gt")
            for c0, cw in ((0, 512), (512, 512), (1024, 1)):
                for f in range(8):
                    gp = psE.tile([128, 512], F32, tag="eng")
                    for t in range(2):
                        nc.tensor.matmul(
                            gp[:, 0:cw],
                            w1_sb[l][:, FF * t + 128 * f:FF * t + 128 * f + 128],
                            (ytA if t == 0 else ytB)[:, c0:c0 + cw],
                            start=(t == 0), stop=(t == 1))
                    nc.scalar.activation(gt[:, S * f + c0:S * f + c0 + cw],
                                         gp[:, 0:cw], AF.Gelu,
                                         bias=b1f_sb[l][:, f:f + 1], scale=1.0)
            for qt_i in range(NQ):
                mw = 128 if qt_i < 8 else 1
                wp = psE.tile([128, E], F32, tag="eng")
                for f in range(8):
                    nc.tensor.matmul(wp[0:mw, :],
                                     gt[:, S * f + 128 * qt_i:
                                        S * f + 128 * qt_i + mw],
                                     w2_sb[l][:, E * f:E * (f + 1)],
                                     start=(f == 0), stop=False)
                nc.tensor.matmul(wp[0:mw, :], ones_row[0:1, 0:mw],
                                 b2r_sb[l], start=False, stop=True)
                sl = slice(E * qt_i, E * (qt_i + 1))
                nc.vector.tensor_tensor(out=h[0:mw, sl], in0=h[0:mw, sl],
                                        in1=wp[0:mw, :], op=OP.add)
                if l < L - 1:
                    # next layer's pos-add + LN1 stats, per-tile (gpsimd add
                    # keeps DVE free; stats follow on DVE)
                    nc.gpsimd.tensor_tensor(out=h[0:mw, sl], in0=h[0:mw, sl],
                                            in1=pos_sb[0:mw, sl], op=OP.add)
                    ln_stats(qt_i)

        # ---------------- classifier ----------------
        hbf = ap_.tile([128, NQ * E], BF16, tag="hbf")
        nc.vector.tensor_copy(hbf[:], h[:])
        mp = psE.tile([1, E], F32, tag="eng")
        for j in range(8):
            nc.tensor.matmul(mp[:], ones_col[:], hbf[:, E * j:E * (j + 1)],
                             start=(j == 0), stop=False)
        nc.tensor.matmul(mp[:], ones_col[0:1, :], hbf[0:1, 8 * E:9 * E],
                         start=False, stop=True)
        pbf = dnp.tile([1, E], BF16, tag="pbf")
        nc.scalar.activation(pbf[:], mp[:], AF.Identity, bias=0.0,
                             scale=1.0 / S)
        # p @ Wc1 + bc1
        ptp_ps = psE.tile([128, 512], BF16, tag="eng")
        pT = dnp.tile([128, 2], BF16, tag="pT")
        for t in range(2):
            nc.tensor.transpose(ptp_ps[:, 128 * t:128 * t + 1],
                                pbf[0:1, 128 * t:128 * (t + 1)],
                                ident[0:1, 0:1])
            nc.vector.tensor_copy(pT[:, t:t + 1],
                                  ptp_ps[:, 128 * t:128 * t + 1])
        c1p = psE.tile([1, E], F32, tag="eng")
        for t in range(2):
            nc.tensor.matmul(c1p[:], pT[:, t:t + 1],
                             wc1_sb[:, E * t:E * (t + 1)],
                             start=(t == 0), stop=False)
        nc.tensor.matmul(c1p[:], ones_row[0:1, 0:1], bc1_sb,
                         start=False, stop=True)
        # LN over the [1, E] row
        p2 = dnp.tile([1, E], F32, tag="p2")
        nc.vector.tensor_copy(p2[:], c1p[:])
        st1 = dnp.tile([1, 6], F32, tag="st1")
        ag1 = dnp.tile([1, 2], F32, tag="ag1")
        nc.vector.bn_stats(st1[:], p2[:])
        nc.vector.bn_aggr(ag1[:], st1[:])
        r1 = dnp.tile([1, 2], F32, tag="r1")
        nc.scalar.activation(r1[:, 0:1], ag1[:, 1:2], AF.Ln,
                             bias=epsb[0:1, 0:1], scale=1.0)
        nc.scalar.activation(r1[:, 0:1], r1[:, 0:1], AF.Exp,
                             bias=0.0, scale=-0.5)
        nc.vector.tensor_scalar(out=p2[:], in0=p2[:], scalar1=ag1[:, 0:1],
                                scalar2=r1[:, 0:1], op0=OP.subtract,
                                op1=OP.mult)
        nc.vector.tensor_tensor(out=p2[:], in0=p2[:], in1=lncg_sb,
                                op=OP.mult)
        nc.vector.tensor_tensor(out=p2[:], in0=p2[:], in1=lncb_sb,
                                op=OP.add)
        p2b = dnp.tile([1, E], BF16, tag="p2b")
        nc.vector.tensor_copy(p2b[:], p2[:])
        p2T = dnp.tile([128, 2], BF16, tag="p2T")
        for t in range(2):
            tp2 = psE.tile([128, 512], BF16, tag="eng")
            nc.tensor.transpose(tp2[:, 0:1], p2b[0:1, 128 * t:128 * (t + 1)],
                                ident[0:1, 0:1])
            nc.vector.tensor_copy(p2T[:, t:t + 1], tp2[:, 0:1])
        op_ = psE.tile([1, NCLS], F32, tag="eng")
        for t in range(2):
            nc.tensor.matmul(op_[:], p2T[:, t:t + 1],
                             wc2_sb[:, NCLS * t:NCLS * (t + 1)],
                             start=(t == 0), stop=False)
        nc.tensor.matmul(op_[:], ones_row[0:1, 0:1], bc2_sb,
                         start=False, stop=True)
        osb = dnp.tile([1, NCLS], F32, tag="osb")
        nc.vector.tensor_copy(osb[:], op_[:])
        nc.sync.dma_start(out_d[:], osb[:])

        for _p in (psE, psV, dn1, dnp, bcp, ptp, ap_, cp):
            _p.release()

    # Steer the ACT-table inserter to the combined ln+exp set: empty the
    # pure-ln / pure-exp sets in the table list it consults (indices must be
    # preserved, so contents are blanked rather than entries removed).
    import concourse.bacc as bacc_mod
    from concourse.hw_specs import get_activation_tables as _gat_orig

    def _gat_patched(arch):
        t = dict(_gat_orig(arch))
        for k in ("natural_log", "exp_and_others", "exp_and_friends"):
            if k in t:
                t[k] = set()
        return t

    bacc_mod.get_activation_tables = _gat_patched
    try:
        nc.compile()
    finally:
        bacc_mod.get_activation_tables = _gat_orig
    return nc


def _prep_shared(inputs):
    """Host-side weight preparation (shared across cores)."""
    bf16 = ml_dtypes.bfloat16
    f32 = np.float32
    g = {k: np.asarray(v, dtype=f32) for k, v in inputs.items()}
    d = {}

    # sequence order [tokens 0..1023, cls] -> permute pos accordingly
    pos_perm = np.concatenate([g["pos"][1:], g["pos"][0:1]], axis=0)
    pos_tm = np.zeros((128, NQ * E), f32)
    for j in range(NQ):
        n = 128 if j < 8 else 1
        pos_tm[0:n, E * j:E * (j + 1)] = pos_perm[128 * j:128 * j + n]

    d["wemb"] = g["W_emb"].astype(bf16)

    perm = np.concatenate([np.arange(32) + 32 * SIG[s] for s in range(8)])

    def pack_k(w):  # [256, X] -> [128, 2X]
        return np.concatenate([w[0:128], w[128:256]], axis=1)

    # rowsb: packed bf16 bias rows on partition 0
    rowsb = np.zeros((1, 2565), f32)
    rowsb[0, RBEMB:RBEMB + E] = g["b_emb"]
    for l in range(L):
        rowsb[0, RWOB + E * l:RWOB + E * (l + 1)] = \
            g["bo"][l] + g["bv"][l] @ g["Wo"][l]
        rowsb[0, RB2R + E * l:RB2R + E * (l + 1)] = g["b2"][l]
    rowsb[0, RBC1:RBC1 + E] = g["bc1"]
    rowsb[0, RBC2:RBC2 + NCLS] = g["bc2"]
    d["rowsb"] = rowsb.astype(bf16)

    rowsf = np.zeros((1, 3 * E), f32)
    rowsf[0, 0:E] = g["cls_token"].reshape(E)
    rowsf[0, E:2 * E] = g["lnc_g"]
    rowsf[0, 2 * E:3 * E] = g["lnc_b"]
    d["rowsf"] = rowsf

    smallf = np.zeros((128, 2896), f32)
    smallf[:, OPOS:OPOS + NQ * E] = pos_tm
    smallf[:, OGEMB:OGEMB + E] = g["g_emb"][None]
    smallf[:, OBEEMB:OBEEMB + E] = g["be_emb"][None]
    for l in range(L):
        o = OSM + 20 * l
        smallf[:, o + 0:o + 2] = pack_k(g["ln1_g"][l].reshape(E, 1))
        smallf[:, o + 2:o + 4] = pack_k(g["ln1_b"][l].reshape(E, 1))
        smallf[:, o + 4:o + 6] = pack_k(g["ln2_g"][l].reshape(E, 1))
        smallf[:, o + 6:o + 8] = pack_k(g["ln2_b"][l].reshape(E, 1))
        smallf[:, o + 8:o + 10] = pack_k(g["bq"][l][perm].reshape(E, 1))
        smallf[:, o + 10:o + 12] = pack_k(g["bk"][l][perm].reshape(E, 1))
        smallf[:, o + 12:o + 20] = np.stack(
            [g["b1"][l][128 * t:128 * (t + 1)] for t in range(8)], axis=1)
    d["smallf"] = smallf

    for l in range(L):
        big = np.zeros((128, 7680), f32)
        big[:, OWQ:OWQ + 2 * E] = pack_k(g["Wq"][l][:, perm])
        big[:, OWK:OWK + 2 * E] = pack_k(g["Wk"][l][:, perm])
        big[:, OWV:OWV + 2 * E] = pack_k(g["Wv"][l][:, perm])
        # wo: slot s cols E*s hold Wo rows of head SIG[s], at partition
        # rows 0..31 for even s and 64..95 for odd s.
        for s in range(8):
            r0 = 0 if s % 2 == 0 else 64
            big[r0:r0 + 32, OWO + E * s:OWO + E * (s + 1)] = \
                g["Wo"][l][32 * SIG[s]:32 * SIG[s] + 32]
        big[:, OW1:OW1 + 2 * FF] = pack_k(g["W1"][l])
        big[:, OW2:OW2 + 8 * E] = np.concatenate(
            [g["W2"][l][128 * t:128 * (t + 1)] for t in range(8)], axis=1)
        d[f"bigw{l}"] = big.astype(bf16)

    wcp = np.zeros((128, 2 * E + 2 * NCLS), f32)
    wcp[:, 0:2 * E] = pack_k(g["Wc1"])
    wcp[:, 2 * E:] = pack_k(g["Wc2"])
    d["wc"] = wcp.astype(bf16)
    return d


def kernel(**inputs):
    sys.path.insert(0, TRN_REPO)
    from concourse.bass_utils import run_bass_kernel_spmd

    if "nc" not in _CACHE:
        _CACHE["nc"] = _build()
    nc = _CACHE["nc"]

    shared = _prep_shared(inputs)
    x = np.asarray(inputs["x"], dtype=np.float32)
    in_maps = [dict(shared, x=np.ascontiguousarray(x[c])) for c in range(B)]
    res = run_bass_kernel_spmd(nc, in_maps, list(range(B)))
    out = np.stack([np.asarray(res.results[c]["out"]).reshape(NCLS)
                    for c in range(B)])
    return out.astype(np.float32)
